# revision 42
# baseline (speedup 1.0000x reference)
"""Causal self-attention (B=2, S=2048, H=1024, NH=16) on 8 TRN2 NeuronCores.

Sharding: core c handles batch b = c//4 and heads [4*(c%4), 4*(c%4)+4).
Tensor-parallel c_attn (column split) AND tensor-parallel c_proj (row
split over this core's 4 head-dims), with a per-chunk bf16 ReduceScatter
over the 4-core batch group producing each core's final output rows.
No gather of O^T and no data-driven offsets are needed.

Single fused software-pipelined stream per core:
  - Attention runs as a flat sequence of (pair, key-block) steps,
    j-chunk outer.  A "pair" is two heads whose Q^T/K^T live on SBUF
    partitions 0-63 (even head) and 64-127 (odd head): their score
    matmuls are issued back-to-back so the PE's 64-row tiling
    (auto-derived tile_position h0/h64) executes them concurrently.
  - exp runs on ACT once per (pair, block) over both heads
    [128, 2, npp] straight out of PSUM (ACT is the critical engine:
    ~80us of exp); scores are issued 2 steps ahead of the matching
    A^T@V so the PE never waits on ACT.
  - The next chunk's x^T transposes (matmul-form: x-block stationary,
    identity streaming - unlike transpose-mode these engage the HAM
    clock un-throttle) and QKV matmuls, and the previous chunk's
    c_proj partial matmuls, are interleaved between attention steps
    as rationed "filler" so the PE stays dense and warm.
  - Softmax normalization per (chunk, pair): ones-column denominators
    from PSUM row 64, PSUM->SBUF reshape DMA, one batched DVE
    reciprocal, one stride-0 broadcast DMA per head, one DVE multiply
    into the pair-packed normalized O^T tile that c_proj consumes.
  - c_proj contracts the core's own 256 head-dims (full-128 matmuls
    over head pairs) per 128-query block; partials go to DRAM in bf16
    and a per-chunk ReduceScatter(add) over the batch group yields the
    core's 128-row slice of each chunk.

All matmul operands bf16 (Wq pre-scaled by 1/8), fp32 PSUM.  b_proj is
added on the host (exact), QKV biases on DVE/GPSIMD during PSUM
evacuation.
"""

import sys

sys.path.insert(0, "/opt/trn_rl_repo")

from collections import deque

import numpy as np

import concourse.bass as bass
import concourse.mybir as mybir
import concourse.tile as tile
from concourse import bacc
from concourse.bass_utils import run_bass_kernel_spmd
from concourse.masks import make_identity

B, S, H, NH, DK = 2, 2048, 1024, 16, 64
NCORES = 8
HPC = 4            # heads per core
CW = HPC * DK      # 256 qkv columns per core
GROUPS = [[0, 1, 2, 3], [4, 5, 6, 7]]

F32 = mybir.dt.float32
BF16 = mybir.dt.bfloat16

DEBUG = False

KT = H // 128   # 8 contraction tiles over H
NQ = S // 512   # 4 query chunks of 512
CK = 512        # chunk width


def _ins_bcast(ap, pos, n):
    """Insert a stride-0 (broadcast) dim of size n at free-dim position pos."""
    a = [list(p) for p in ap.ap]
    return bass.AP(tensor=ap.tensor, offset=ap.offset,
                   ap=a[:pos] + [[0, n]] + a[pos:])


def _bcast_rows(src_ap, parts):
    """Stride-0 partition broadcast: repeat src_ap's whole AP across parts."""
    a = [list(p) for p in src_ap.ap]
    return bass.AP(tensor=src_ap.tensor, offset=src_ap.offset,
                   ap=[[0, parts]] + a)


def build_nc():
    nc = bacc.Bacc(None, target_bir_lowering=False, debug=False,
                   num_devices=NCORES)

    xb = nc.declare_dram_parameter("xb", [S, H], BF16, isOutput=False)
    wq = nc.declare_dram_parameter("wq", [H, CW], BF16, isOutput=False)
    wk = nc.declare_dram_parameter("wk", [H, CW], BF16, isOutput=False)
    wv = nc.declare_dram_parameter("wv", [H, CW], BF16, isOutput=False)
    # c_proj rows for this core's 4 heads, pair-packed:
    # wp[d, p*H + n] = w_proj[h0*64 + 128*p + d, n]
    wp = nc.declare_dram_parameter("wp", [128, 2 * H], BF16, isOutput=False)
    bq = nc.declare_dram_parameter("bq", [CW], F32, isOutput=False)
    bk = nc.declare_dram_parameter("bk", [CW], F32, isOutput=False)
    bv = nc.declare_dram_parameter("bv", [CW], F32, isOutput=False)
    outs = [nc.declare_dram_parameter(f"out{j}", [128, H], BF16, isOutput=True)
            for j in range(NQ)]
    dbg = {}
    if DEBUG:
        for nm, shape, dt in [
            ("dbg_qt", [128, 2 * CK], BF16), ("dbg_kt", [128, 2 * CK], BF16),
            ("dbg_v4", [128, 4 * HPC * (DK + 1)], BF16),
            ("dbg_comb", [128, CK], BF16), ("dbg_srow", [2, CK], BF16),
            ("dbg_rbc", [128, CK], BF16), ("dbg_no", [128, CK], BF16),
            ("dbg_no2", [128, CK], BF16), ("dbg_partial", [CK, H], BF16),
            ("dbg_ag", [128, 2 * CK], BF16),
        ]:
            dbg[nm] = nc.declare_dram_parameter(nm, shape, dt, isOutput=True)

    with tile.TileContext(nc) as tc:
        with (
            tc.tile_pool(name="dram", bufs=1, space="DRAM") as dram,
            tc.tile_pool(name="psum", bufs=1, space="PSUM") as psum,
            tc.tile_pool(name="persist", bufs=1) as pw,
            tc.tile_pool(name="work", bufs=1) as pa,
        ):
            partial = [dram.tile([CK, H], BF16, name=f"partial{j}")
                       for j in range(NQ)]
            rsout = [dram.tile([128, H], BF16, name=f"rsout{j}")
                     for j in range(NQ)]
            # DRAM bounce rows for the reciprocal broadcast (stride-0
            # partition-source DMA is only legal from DRAM)
            rsumd = [dram.tile([2, CK], BF16, name=f"rsumd{i}")
                     for i in range(4)]
            # contiguous per-column-half tiles for the last chunk's split RS
            partial3h = [dram.tile([CK, 512], BF16, name=f"partial3h{h}")
                         for h in range(2)]
            rsout3h = [dram.tile([128, 512], BF16, name=f"rsout3h{h}")
                       for h in range(2)]

            ident = pw.tile([128, 128], BF16)
            ident_f32 = pw.tile([128, 128], F32)
            make_identity(nc, ident_f32)
            nc.vector.tensor_copy(ident, ident_f32)
            # lower-triangle-in-q mask: tri[k, q] = 1 if q >= k else 0
            tri_f32 = pw.tile([128, 128], F32)
            nc.gpsimd.memset(tri_f32, 1.0)
            nc.gpsimd.affine_select(
                out=tri_f32, in_=tri_f32, compare_op=mybir.AluOpType.is_ge,
                fill=0.0, base=0, pattern=[[1, 128]], channel_multiplier=-1)
            tri = pw.tile([128, 128], BF16)
            nc.vector.tensor_copy(tri, tri_f32)

            # weights: [128, k-tile, cols]
            wq_sb = pw.tile([128, KT, CW], BF16)
            wk_sb = pw.tile([128, KT, CW], BF16)
            wv_sb = pw.tile([128, KT, CW], BF16)
            nc.gpsimd.dma_start(
                out=wq_sb, in_=wq.ap().rearrange("(k p) c -> p k c", p=128))
            nc.gpsimd.dma_start(
                out=wk_sb, in_=wk.ap().rearrange("(k p) c -> p k c", p=128))
            nc.gpsimd.dma_start(
                out=wv_sb, in_=wv.ap().rearrange("(k p) c -> p k c", p=128))
            wp_sb = pw.tile([128, 2, H], BF16)
            nc.gpsimd.dma_start(
                out=wp_sb, in_=wp.ap().rearrange("p (r c) -> p r c", r=2))

            bq_sb = pw.tile([128, 2], F32)
            bk_sb = pw.tile([128, 2], F32)
            nc.gpsimd.dma_start(out=bq_sb,
                                in_=bq.ap().rearrange("(h p) -> p h", p=128))
            nc.gpsimd.dma_start(out=bk_sb,
                                in_=bk.ap().rearrange("(h p) -> p h", p=128))
            bv_bc = pw.tile([128, CW], F32)
            nc.gpsimd.dma_start(out=bv_bc, in_=_bcast_rows(bv.ap(), 128))

            # tiny warmup collective: absorbs the ~11us first-collective
            # trigger delay during the prologue
            warm_in = dram.tile([4, H], BF16, name="warm_in")
            warm_out = dram.tile([1, H], BF16, name="warm_out")
            nc.gpsimd.collective_compute(
                "ReduceScatter", mybir.AluOpType.add,
                replica_groups=GROUPS,
                ins=[warm_in.opt()], outs=[warm_out.opt()])

            # per-chunk tensors (separate tiles -> no false dependencies)
            xT = [pw.tile([128, KT, CK], BF16, name=f"xT{j}")
                  for j in range(NQ)]
            QTt = [pw.tile([128, 2, CK], BF16, name=f"QT{j}")
                   for j in range(NQ)]
            KTt = [pw.tile([128, 2, CK], BF16, name=f"KT{j}")
                   for j in range(NQ)]
            V4 = [pw.tile([128, 4, HPC, DK + 1], BF16, name=f"V4{j}")
                  for j in range(NQ)]
            for j in range(NQ):
                nc.gpsimd.memset(V4[j], 1.0)  # ones col (rest overwritten)

            # ---------------- filler unit machinery ----------------
            # Each unit: (kind, chunk, cost_ns, closure).  kinds 't'/'q'
            # must be flushed before the first attention step of their
            # chunk; 'c' (c_proj) units are flexible.
            mq = deque()
            oq = deque()
            def tr_evac(dst, src):
                nc.vector.tensor_copy(dst, src)

            def push_transpose_chunk(j):
                """x^T for chunk j: 4 xs DMAs now, 8 matmul-form units."""
                xs_tiles = []
                for sl in range(4):
                    si = 4 * j + sl
                    xs = pa.tile([128, H], BF16, tag="xs", bufs=4,
                                 name=f"xs{si}")
                    nc.sync.dma_start(out=xs,
                                      in_=xb[si * 128:(si + 1) * 128, :])
                    xs_tiles.append(xs)
                for sl in range(4):
                    for kh in range(2):
                        def tunit(j=j, sl=sl, kh=kh, xs=xs_tiles[sl]):
                            ptp = psum.tile([128, 512], F32, tag="pf", bufs=2,
                                            name=f"ptp{j}_{sl}_{kh}")
                            for i in range(4):
                                k = 4 * kh + i
                                nc.tensor.matmul(
                                    ptp[:, i * 128:(i + 1) * 128],
                                    xs[:, k * 128:(k + 1) * 128],
                                    ident, start=True, stop=True)
                            src = ptp[:, :].rearrange("p (k f) -> p k f", k=4)
                            dst = xT[j][:, 4 * kh:4 * kh + 4,
                                        sl * 128:(sl + 1) * 128]
                            tr_evac(dst, src)
                        mq.append((350, tunit))

            def push_qkv_chunk(j):
                for ti, (wt, dst, bias) in enumerate(
                        ((wq_sb, QTt[j], bq_sb), (wk_sb, KTt[j], bk_sb))):
                    for half in range(2):
                        pqs = {}

                        def qunit_a(j=j, wt=wt, half=half, pqs=pqs, ti=ti):
                            pq = psum.tile([128, 512], F32, tag="pf", bufs=2,
                                           name=f"pq{j}_{ti}_{half}")
                            pqs['t'] = pq
                            for k in range(4):
                                nc.tensor.matmul(
                                    pq,
                                    wt[:, k, half * 128:(half + 1) * 128],
                                    xT[j][:, k, :],
                                    start=(k == 0), stop=False)

                        def qunit_b(j=j, ti=ti, wt=wt, dst=dst, bias=bias,
                                    half=half, pqs=pqs):
                            pq = pqs.pop('t')
                            for k in range(4, KT):
                                nc.tensor.matmul(
                                    pq,
                                    wt[:, k, half * 128:(half + 1) * 128],
                                    xT[j][:, k, :],
                                    start=False, stop=(k == KT - 1))
                            if ti == 0:
                                nc.scalar.activation(
                                    dst[:, half, :], pq,
                                    mybir.ActivationFunctionType.Identity,
                                    bias=bias[:, half:half + 1])
                            else:
                                nc.vector.tensor_scalar_add(
                                    dst[:, half, :], pq,
                                    bias[:, half:half + 1])
                        mq.append((900, qunit_a))
                        mq.append((900, qunit_b))
                for sp in range(2):  # si pairs (0,1), (2,3)
                    pvs = {}

                    def vunit_a(j=j, sp=sp, pvs=pvs):
                        pv = psum.tile([128, 512], F32, tag="pf", bufs=2,
                                       name=f"pv{j}_{sp}")
                        pvs['t'] = pv
                        sl = 2 * sp
                        for k in range(KT):
                            nc.tensor.matmul(
                                pv[:, 0:256],
                                xT[j][:, k, sl * 128:(sl + 1) * 128],
                                wv_sb[:, k, :],
                                start=(k == 0), stop=(k == KT - 1))

                    def vunit_b(j=j, sp=sp, pvs=pvs):
                        pv = pvs.pop('t')
                        sl = 2 * sp + 1
                        for k in range(KT):
                            nc.tensor.matmul(
                                pv[:, 256:512],
                                xT[j][:, k, sl * 128:(sl + 1) * 128],
                                wv_sb[:, k, :],
                                start=(k == 0), stop=(k == KT - 1))
                        pv_v = pv[:, :].rearrange("p (s h d) -> p s h d",
                                                  s=2, h=HPC)
                        bv_h = bv_bc[:, :].rearrange("p (h d) -> p h d",
                                                     h=HPC)
                        nc.vector.tensor_add(
                            V4[j][:, 2 * sp:2 * sp + 2, :, 0:DK],
                            pv_v, _ins_bcast(bv_h, 1, 2))
                    mq.append((900, vunit_a))
                    mq.append((900, vunit_b))

            def push_cproj_chunk(j, normo):
                last = j == NQ - 1
                no = {}

                def nunit(j=j, normo=normo, no=no):
                    # all normalize multiplies for the chunk in one burst;
                    # by pop time the reciprocal chain has long drained
                    for qb in range(4):
                        qs = slice(qb * 128, (qb + 1) * 128)
                        no[qb] = [
                            pa.tile([128, 128], BF16, tag=f"no{p}",
                                    bufs=8, name=f"no{p}_{j}_{qb}")
                            for p in range(2)]
                        for p in range(2):
                            cbx, rbc = normo[p]
                            nc.vector.tensor_mul(
                                no[qb][p][0:64, :], cbx[0][0:64, qs],
                                rbc[0][:, qs])
                            nc.vector.tensor_mul(
                                no[qb][p][64:128, :], cbx[1][0:64, qs],
                                rbc[1][:, qs])
                oq.append((step_no[0] + 8, 300, nunit))
                for qb in range(4):
                    for nh in range(2):
                        def cunit(j=j, qb=qb, nh=nh, last=last, no=no):
                            pt = psum.tile([128, 512], F32, tag="pf", bufs=2,
                                           name=f"pt{j}_{qb}_{nh}")
                            for p in range(2):
                                nc.tensor.matmul(
                                    pt,
                                    no[qb][p][:, :],
                                    wp_sb[:, p, nh * 512:(nh + 1) * 512],
                                    start=(p == 0), stop=(p == 1))
                            ps = pa.tile([128, 512], BF16, tag="ps", bufs=3,
                                         name=f"ps{j}_{qb}_{nh}")
                            if last:
                                nc.scalar.copy(ps, pt)
                            else:
                                nc.vector.tensor_copy(ps, pt)
                            nc.sync.dma_start(
                                out=partial[j][qb * 128:(qb + 1) * 128,
                                               nh * 512:(nh + 1) * 512],
                                in_=ps)
                            if qb == 3 and nh == 1:
                                nc.gpsimd.collective_compute(
                                    "ReduceScatter", mybir.AluOpType.add,
                                    replica_groups=GROUPS,
                                    ins=[partial[j].opt()],
                                    outs=[rsout[j].opt()])
                                nc.sync.dma_start(out=outs[j][:, :],
                                                  in_=rsout[j][:, :])
                        oq.append((step_no[0] + 9, 500, cunit))

            # ---------------- attention step machinery ----------------
            normo_h = {}   # (j, pair) -> [even-head tile rows 0-63, odd 64-127]
            pav_h = {}     # (j, pair) -> (pav_e, pav_o)

            def emit_scores(j, pair, ki):
                jk, kb = divmod(ki, 4)
                off = max(0, 128 * ki - CK * j)
                npp = CK - off
                sg = psum.tile([128, 2, CK], F32, tag="sg", bufs=2,
                               name=f"sg{j}_{pair}_{ki}")
                for half in range(2):
                    base = 64 * half
                    nc.tensor.matmul(
                        sg[:, half, off:CK],
                        KTt[jk][base:base + DK, pair,
                                kb * 128:(kb + 1) * 128],
                        QTt[j][base:base + DK, pair, off:CK],
                        start=True, stop=True)
                ag = pa.tile([128, 2, CK], BF16, tag="A", bufs=4,
                             name=f"A{j}_{pair}_{ki}")
                nc.scalar.activation(
                    ag[:, :, off:CK], sg[:, :, off:CK],
                    mybir.ActivationFunctionType.Exp)
                if ki >= 4 * j:  # diagonal block: causal corner mask
                    av = ag[:, :, off:off + 128]
                    nc.vector.tensor_mul(av, av, _ins_bcast(tri[:, :], 1, 2))
                if DEBUG and (j, pair, ki) == (0, 0, 0):
                    nc.sync.dma_start(
                        out=dbg["dbg_ag"].ap().rearrange(
                            "p (h c) -> p h c", h=2),
                        in_=ag)
                return (j, pair, ki, ag, off)

            def emit_av(st):
                j, pair, ki, ag, off = st
                nblk = 4 * j + 4
                if ki == 0:
                    pav_h[(j, pair)] = tuple(
                        psum.tile([65, 512], F32, tag="pav", bufs=2,
                                  name=f"pav{j}_{pair}_{h}")
                        for h in range(2))
                jk, kb = divmod(ki, 4)
                for half in range(2):
                    nc.tensor.matmul(
                        pav_h[(j, pair)][half][:, off:CK],
                        V4[jk][:, kb, 2 * pair + half, :],
                        ag[:, half, off:CK],
                        start=(ki == 0), stop=(ki == nblk - 1))
                if ki == nblk - 1:
                    emit_norm(j, pair)

            def emit_norm(j, pair):
                pav_e, pav_o = pav_h.pop((j, pair))
                # [0:65] copies carry the denominator row along with O^T,
                # releasing the pav PSUM banks in two DVE ops.
                cbx = [pa.tile([65, CK], BF16, tag=f"cbx{h}", bufs=2,
                               name=f"cbx{h}_{j}_{pair}") for h in range(2)]
                cpy = nc.scalar.copy if j == NQ - 1 else nc.vector.tensor_copy
                cpy(cbx[0][0:65, :], pav_e[0:65, :])
                cpy(cbx[1][0:65, :], pav_o[0:65, :])
                srec = pa.tile([128, 2, 4], BF16, tag="srec", bufs=2,
                               name=f"srec{j}_{pair}")
                nc.sync.dma_start(out=srec[:, 0, :], in_=cbx[0][64:65, :])
                nc.sync.dma_start(out=srec[:, 1, :], in_=cbx[1][64:65, :])
                with nc.allow_low_precision(
                        reason="bf16 recip of O(1e2) softmax sums"):
                    nc.vector.reciprocal(srec, srec)
                srd = rsumd[(j * 2 + pair) % 4]
                nc.sync.dma_start(out=srd[0:1, :], in_=srec[:, 0, :])
                nc.sync.dma_start(out=srd[1:2, :], in_=srec[:, 1, :])
                rbc = [pa.tile([64, CK], BF16, tag=f"rbc{h}", bufs=2,
                               name=f"rbc{h}_{j}_{pair}") for h in range(2)]
                nc.gpsimd.dma_start(out=rbc[0][0:64, :],
                                    in_=_bcast_rows(srd[0, :], 64))
                nc.gpsimd.dma_start(out=rbc[1][0:64, :],
                                    in_=_bcast_rows(srd[1, :], 64))
                # the normalize multiplies happen lazily inside the c_proj
                # units, long after this chain has drained
                normo_h[(j, pair)] = (cbx, rbc)
                if DEBUG and j == 0 and pair == 0:
                    nc.sync.dma_start(out=dbg["dbg_comb"][0:64, :],
                                      in_=cbx[0][0:64, :])
                    nc.sync.dma_start(out=dbg["dbg_comb"][64:128, :],
                                      in_=cbx[1][0:64, :])
                    nc.sync.dma_start(out=dbg["dbg_srow"][0:1, :],
                                      in_=cbx[0][64:65, :])
                    nc.sync.dma_start(out=dbg["dbg_srow"][1:2, :],
                                      in_=cbx[1][64:65, :])
                    nc.sync.dma_start(out=dbg["dbg_rbc"][0:64, :], in_=rbc[0])
                    nc.sync.dma_start(out=dbg["dbg_rbc"][64:128, :],
                                      in_=rbc[1])
                    nc.sync.dma_start(out=dbg["dbg_no"][:, :], in_=no)
                if DEBUG and j == 2 and pair == 0:
                    nc.sync.dma_start(out=dbg["dbg_no2"][:, :], in_=no)
                if pair == 1:
                    push_cproj_chunk(j, [normo_h.pop((j, 0)),
                                         normo_h.pop((j, 1))])

            # ---------------- the fused stream ----------------
            steps = [(j, pair, ki)
                     for j in range(NQ)
                     for pair in range(2)
                     for ki in range(4 * j + 4)]

            push_transpose_chunk(0)
            push_qkv_chunk(0)

            pending = deque()   # scores emitted, AV not yet (depth 2)
            cur_chunk = [-1]
            step_no = [0]
            mquota = [0.0]
            mbudget = [0.0]

            for (j, pair, ki) in steps:
                if j != cur_chunk[0]:
                    cur_chunk[0] = j
                    # the current chunk's x^T/QKV must be fully emitted
                    # before its first scores instruction
                    while mq:
                        cost, fn = mq.popleft()
                        fn()
                    if j + 1 < NQ:
                        push_transpose_chunk(j + 1)
                        push_qkv_chunk(j + 1)
                    # ration the next chunk's mandatory work over this
                    # chunk's steps (front-loaded slightly: x0.75 steps)
                    nsteps = 2 * (4 * j + 4)
                    mquota[0] = sum(u[0] for u in mq) / max(nsteps * 0.75, 1)
                    mbudget[0] = 0.0
                pending.append(emit_scores(j, pair, ki))
                mbudget[0] += mquota[0]
                while mq and mbudget[0] >= mq[0][0] * 0.5:
                    cost, fn = mq.popleft()
                    mbudget[0] -= cost
                    fn()
                step_no[0] += 1
                if oq and step_no[0] >= oq[0][0]:
                    # trickle one c_proj unit per step, delayed so the
                    # norm chain has completed by the first pop
                    _, cost, fn = oq.popleft()
                    fn()
                if len(pending) > 2:
                    emit_av(pending.popleft())

            while pending:
                emit_av(pending.popleft())
            while mq:
                cost, fn = mq.popleft()
                fn()
            while oq:
                _, cost, fn = oq.popleft()
                fn()

            if DEBUG:
                nc.sync.dma_start(
                    out=dbg["dbg_qt"].ap().rearrange("p (h c) -> p h c", h=2),
                    in_=QTt[0])
                nc.sync.dma_start(
                    out=dbg["dbg_kt"].ap().rearrange("p (h c) -> p h c", h=2),
                    in_=KTt[0])
                nc.sync.dma_start(
                    out=dbg["dbg_v4"].ap().rearrange(
                        "p (s h d) -> p s h d", s=4, h=HPC),
                    in_=V4[0])
                nc.sync.dma_start(out=dbg["dbg_partial"][:, :],
                                  in_=partial[0][:, :])

    nc.compile()
    return nc


def make_in_maps(x, w_attn, b_attn, w_proj):
    import ml_dtypes
    bf = ml_dtypes.bfloat16
    x = np.asarray(x, dtype=np.float32)
    w_attn = np.asarray(w_attn, dtype=np.float32)
    b_attn = np.asarray(b_attn, dtype=np.float32)
    w_proj = np.asarray(w_proj, dtype=np.float32)
    in_maps = []
    for c in range(NCORES):
        b, g = divmod(c, 4)
        h0 = g * HPC
        cs = slice(h0 * DK, h0 * DK + CW)
        wpc = np.ascontiguousarray(w_proj[cs, :])          # [256, H]
        wp_pair = np.ascontiguousarray(
            wpc.reshape(2, 128, H).transpose(1, 0, 2).reshape(128, 2 * H))
        in_maps.append({
            "xb": np.ascontiguousarray(x[b]).astype(bf),
            # fold the 1/sqrt(DK)=2^-3 score scale into Wq/bq (exact in fp32)
            "wq": (np.ascontiguousarray(w_attn[:, cs])
                   * np.float32(0.125)).astype(bf),
            "wk": np.ascontiguousarray(w_attn[:, H:][:, cs]).astype(bf),
            "wv": np.ascontiguousarray(w_attn[:, 2 * H:][:, cs]).astype(bf),
            "wp": wp_pair.astype(bf),
            "bq": np.ascontiguousarray(b_attn[cs]) * np.float32(0.125),
            "bk": np.ascontiguousarray(b_attn[H:][cs]),
            "bv": np.ascontiguousarray(b_attn[2 * H:][cs]),
        })
    return in_maps


_NC = None


def kernel(x, w_attn, b_attn, w_proj, b_proj):
    global _NC
    if _NC is None:
        _NC = build_nc()

    b_proj = np.asarray(b_proj, dtype=np.float32)
    in_maps = make_in_maps(x, w_attn, b_attn, w_proj)
    res = run_bass_kernel_spmd(_NC, in_maps, core_ids=list(range(NCORES)))

    outp = np.empty((B, S, H), dtype=np.float32)
    for c in range(NCORES):
        b, g = divmod(c, 4)
        for j in range(NQ):
            o = res.results[c][f"out{j}"].astype(np.float32)
            outp[b, j * CK + g * 128: j * CK + (g + 1) * 128, :] = o
    outp += b_proj  # row-broadcast add, exact
    return outp


# revision 43
# speedup vs baseline: 1.0854x; 1.0854x over previous
"""Causal self-attention (B=2, S=2048, H=1024, NH=16) on 8 TRN2 NeuronCores.

Sharding: core c handles batch b = c//4 and heads [4*(c%4), 4*(c%4)+4).
Tensor-parallel c_attn (column split) AND tensor-parallel c_proj (row
split over this core's 4 head-dims), with a per-chunk bf16 ReduceScatter
over the 4-core batch group producing each core's final output rows.
No gather of O^T and no data-driven offsets are needed.

Single fused software-pipelined stream per core:
  - Attention runs as a flat sequence of (pair, key-block) steps,
    j-chunk outer.  A "pair" is two heads whose Q^T/K^T live on SBUF
    partitions 0-63 (even head) and 64-127 (odd head): their score
    matmuls are issued back-to-back so the PE's 64-row tiling
    (auto-derived tile_position h0/h64) executes them concurrently.
  - exp runs on ACT once per (pair, block) over both heads
    [128, 2, npp] straight out of PSUM (ACT is the critical engine:
    ~80us of exp); scores are issued 2 steps ahead of the matching
    A^T@V so the PE never waits on ACT.
  - The next chunk's x^T transposes (matmul-form: x-block stationary,
    identity streaming - unlike transpose-mode these engage the HAM
    clock un-throttle) and QKV matmuls, and the previous chunk's
    c_proj partial matmuls, are interleaved between attention steps
    as rationed "filler" so the PE stays dense and warm.
  - Softmax normalization per (chunk, pair): ones-column denominators
    from PSUM row 64, PSUM->SBUF reshape DMA, one batched DVE
    reciprocal, one stride-0 broadcast DMA per head, one DVE multiply
    into the pair-packed normalized O^T tile that c_proj consumes.
  - c_proj contracts the core's own 256 head-dims (full-128 matmuls
    over head pairs) per 128-query block; partials go to DRAM in bf16
    and a per-chunk ReduceScatter(add) over the batch group yields the
    core's 128-row slice of each chunk.

All matmul operands bf16 (Wq pre-scaled by 1/8), fp32 PSUM.  b_proj is
added on the host (exact), QKV biases on DVE/GPSIMD during PSUM
evacuation.
"""

import sys

sys.path.insert(0, "/opt/trn_rl_repo")

from collections import deque

import numpy as np

import concourse.bass as bass
import concourse.mybir as mybir
import concourse.tile as tile
from concourse import bacc
from concourse.bass_utils import run_bass_kernel_spmd
from concourse.masks import make_identity

B, S, H, NH, DK = 2, 2048, 1024, 16, 64
NCORES = 8
HPC = 4            # heads per core
CW = HPC * DK      # 256 qkv columns per core
GROUPS = [[0, 1, 2, 3], [4, 5, 6, 7]]

F32 = mybir.dt.float32
BF16 = mybir.dt.bfloat16

DEBUG = False

KT = H // 128   # 8 contraction tiles over H
NQ = S // 512   # 4 query chunks of 512
CK = 512        # chunk width


def _ins_bcast(ap, pos, n):
    """Insert a stride-0 (broadcast) dim of size n at free-dim position pos."""
    a = [list(p) for p in ap.ap]
    return bass.AP(tensor=ap.tensor, offset=ap.offset,
                   ap=a[:pos] + [[0, n]] + a[pos:])


def _bcast_rows(src_ap, parts):
    """Stride-0 partition broadcast: repeat src_ap's whole AP across parts."""
    a = [list(p) for p in src_ap.ap]
    return bass.AP(tensor=src_ap.tensor, offset=src_ap.offset,
                   ap=[[0, parts]] + a)


def build_nc():
    nc = bacc.Bacc(None, target_bir_lowering=False, debug=False,
                   num_devices=NCORES)

    xb = nc.declare_dram_parameter("xb", [S, H], BF16, isOutput=False)
    wq = nc.declare_dram_parameter("wq", [H, CW], BF16, isOutput=False)
    wk = nc.declare_dram_parameter("wk", [H, CW], BF16, isOutput=False)
    wv = nc.declare_dram_parameter("wv", [H, CW], BF16, isOutput=False)
    # c_proj rows for this core's 4 heads, pair-packed:
    # wp[d, p*H + n] = w_proj[h0*64 + 128*p + d, n]
    wp = nc.declare_dram_parameter("wp", [128, 2 * H], BF16, isOutput=False)
    bq = nc.declare_dram_parameter("bq", [CW], F32, isOutput=False)
    bk = nc.declare_dram_parameter("bk", [CW], F32, isOutput=False)
    bv = nc.declare_dram_parameter("bv", [CW], F32, isOutput=False)
    outs = [nc.declare_dram_parameter(f"out{j}", [128, H], BF16, isOutput=True)
            for j in range(NQ)]
    dbg = {}
    if DEBUG:
        for nm, shape, dt in [
            ("dbg_qt", [128, 2 * CK], BF16), ("dbg_kt", [128, 2 * CK], BF16),
            ("dbg_v4", [128, 4 * HPC * (DK + 1)], BF16),
            ("dbg_comb", [128, CK], BF16), ("dbg_srow", [2, CK], BF16),
            ("dbg_rbc", [128, CK], BF16), ("dbg_no", [128, CK], BF16),
            ("dbg_no2", [128, CK], BF16), ("dbg_partial", [CK, H], BF16),
            ("dbg_ag", [128, 2 * CK], BF16),
        ]:
            dbg[nm] = nc.declare_dram_parameter(nm, shape, dt, isOutput=True)

    with tile.TileContext(nc) as tc:
        with (
            tc.tile_pool(name="dram", bufs=1, space="DRAM") as dram,
            tc.tile_pool(name="psum", bufs=1, space="PSUM") as psum,
            tc.tile_pool(name="persist", bufs=1) as pw,
            tc.tile_pool(name="work", bufs=1) as pa,
        ):
            partial = [dram.tile([CK, H], BF16, name=f"partial{j}")
                       for j in range(NQ)]
            rsout = [dram.tile([128, H], BF16, name=f"rsout{j}")
                     for j in range(NQ)]
            # DRAM bounce rows for the reciprocal broadcast (stride-0
            # partition-source DMA is only legal from DRAM)
            rsumd = [dram.tile([2, CK], BF16, name=f"rsumd{i}")
                     for i in range(4)]
            # contiguous per-column-half tiles for the last chunk's split RS
            partial3h = [dram.tile([CK, 512], BF16, name=f"partial3h{h}")
                         for h in range(2)]
            rsout3h = [dram.tile([128, 512], BF16, name=f"rsout3h{h}")
                       for h in range(2)]

            ident = pw.tile([128, 128], BF16)
            ident_f32 = pw.tile([128, 128], F32)
            make_identity(nc, ident_f32)
            nc.vector.tensor_copy(ident, ident_f32)
            # lower-triangle-in-q mask: tri[k, q] = 1 if q >= k else 0
            tri_f32 = pw.tile([128, 128], F32)
            nc.gpsimd.memset(tri_f32, 1.0)
            nc.gpsimd.affine_select(
                out=tri_f32, in_=tri_f32, compare_op=mybir.AluOpType.is_ge,
                fill=0.0, base=0, pattern=[[1, 128]], channel_multiplier=-1)
            tri = pw.tile([128, 128], BF16)
            nc.vector.tensor_copy(tri, tri_f32)

            # weights: [128, k-tile, cols]
            wq_sb = pw.tile([128, KT, CW], BF16)
            wk_sb = pw.tile([128, KT, CW], BF16)
            wv_sb = pw.tile([128, KT, CW], BF16)
            nc.gpsimd.dma_start(
                out=wq_sb, in_=wq.ap().rearrange("(k p) c -> p k c", p=128))
            nc.gpsimd.dma_start(
                out=wk_sb, in_=wk.ap().rearrange("(k p) c -> p k c", p=128))
            nc.gpsimd.dma_start(
                out=wv_sb, in_=wv.ap().rearrange("(k p) c -> p k c", p=128))
            wp_sb = pw.tile([128, 2, H], BF16)
            nc.gpsimd.dma_start(
                out=wp_sb, in_=wp.ap().rearrange("p (r c) -> p r c", r=2))

            bq_sb = pw.tile([128, 2], F32)
            bk_sb = pw.tile([128, 2], F32)
            nc.gpsimd.dma_start(out=bq_sb,
                                in_=bq.ap().rearrange("(h p) -> p h", p=128))
            nc.gpsimd.dma_start(out=bk_sb,
                                in_=bk.ap().rearrange("(h p) -> p h", p=128))
            bv_bc = pw.tile([128, CW], F32)
            nc.gpsimd.dma_start(out=bv_bc, in_=_bcast_rows(bv.ap(), 128))

            # tiny warmup collective: absorbs the ~11us first-collective
            # trigger delay during the prologue
            warm_in = dram.tile([4, H], BF16, name="warm_in")
            warm_out = dram.tile([1, H], BF16, name="warm_out")
            nc.gpsimd.collective_compute(
                "ReduceScatter", mybir.AluOpType.add,
                replica_groups=GROUPS,
                ins=[warm_in.opt()], outs=[warm_out.opt()])

            # per-chunk tensors (separate tiles -> no false dependencies)
            xT = [pw.tile([128, KT, CK], BF16, name=f"xT{j}")
                  for j in range(NQ)]
            QTt = [pw.tile([128, 2, CK], BF16, name=f"QT{j}")
                   for j in range(NQ)]
            KTt = [pw.tile([128, 2, CK], BF16, name=f"KT{j}")
                   for j in range(NQ)]
            V4 = [pw.tile([128, 4, HPC, DK + 1], BF16, name=f"V4{j}")
                  for j in range(NQ)]
            for j in range(NQ):
                nc.gpsimd.memset(V4[j], 1.0)  # ones col (rest overwritten)

            # ---------------- filler unit machinery ----------------
            # Each unit: (kind, chunk, cost_ns, closure).  kinds 't'/'q'
            # must be flushed before the first attention step of their
            # chunk; 'c' (c_proj) units are flexible.
            mq = deque()
            oq = deque()
            def tr_evac(dst, src):
                nc.vector.tensor_copy(dst, src)

            def push_transpose_chunk(j):
                """x^T for chunk j: 4 xs DMAs now, 8 matmul-form units."""
                xs_tiles = []
                for sl in range(4):
                    si = 4 * j + sl
                    xs = pa.tile([128, H], BF16, tag="xs", bufs=4,
                                 name=f"xs{si}")
                    nc.sync.dma_start(out=xs,
                                      in_=xb[si * 128:(si + 1) * 128, :])
                    xs_tiles.append(xs)
                for sl in range(4):
                    for kh in range(2):
                        def tunit(j=j, sl=sl, kh=kh, xs=xs_tiles[sl]):
                            ptp = psum.tile([128, 512], F32, tag="pf", bufs=2,
                                            name=f"ptp{j}_{sl}_{kh}")
                            for i in range(4):
                                k = 4 * kh + i
                                nc.tensor.matmul(
                                    ptp[:, i * 128:(i + 1) * 128],
                                    xs[:, k * 128:(k + 1) * 128],
                                    ident, start=True, stop=True)
                            src = ptp[:, :].rearrange("p (k f) -> p k f", k=4)
                            dst = xT[j][:, 4 * kh:4 * kh + 4,
                                        sl * 128:(sl + 1) * 128]
                            tr_evac(dst, src)
                        mq.append((350, tunit))

            def push_qkv_chunk(j):
                for ti, (wt, dst, bias) in enumerate(
                        ((wq_sb, QTt[j], bq_sb), (wk_sb, KTt[j], bk_sb))):
                    for half in range(2):
                        pqs = {}

                        def qunit_a(j=j, wt=wt, half=half, pqs=pqs, ti=ti):
                            pq = psum.tile([128, 512], F32, tag="pf", bufs=2,
                                           name=f"pq{j}_{ti}_{half}")
                            pqs['t'] = pq
                            for k in range(4):
                                nc.tensor.matmul(
                                    pq,
                                    wt[:, k, half * 128:(half + 1) * 128],
                                    xT[j][:, k, :],
                                    start=(k == 0), stop=False)

                        def qunit_b(j=j, ti=ti, wt=wt, dst=dst, bias=bias,
                                    half=half, pqs=pqs):
                            pq = pqs.pop('t')
                            for k in range(4, KT):
                                nc.tensor.matmul(
                                    pq,
                                    wt[:, k, half * 128:(half + 1) * 128],
                                    xT[j][:, k, :],
                                    start=False, stop=(k == KT - 1))
                            if ti == 0:
                                nc.scalar.activation(
                                    dst[:, half, :], pq,
                                    mybir.ActivationFunctionType.Identity,
                                    bias=bias[:, half:half + 1])
                            else:
                                nc.vector.tensor_scalar_add(
                                    dst[:, half, :], pq,
                                    bias[:, half:half + 1])
                        mq.append((900, qunit_a))
                        mq.append((900, qunit_b))
                for sp in range(2):  # si pairs (0,1), (2,3)
                    pvs = {}

                    def vunit_a(j=j, sp=sp, pvs=pvs):
                        pv = psum.tile([128, 512], F32, tag="pf", bufs=2,
                                       name=f"pv{j}_{sp}")
                        pvs['t'] = pv
                        sl = 2 * sp
                        for k in range(KT):
                            nc.tensor.matmul(
                                pv[:, 0:256],
                                xT[j][:, k, sl * 128:(sl + 1) * 128],
                                wv_sb[:, k, :],
                                start=(k == 0), stop=(k == KT - 1))

                    def vunit_b(j=j, sp=sp, pvs=pvs):
                        pv = pvs.pop('t')
                        sl = 2 * sp + 1
                        for k in range(KT):
                            nc.tensor.matmul(
                                pv[:, 256:512],
                                xT[j][:, k, sl * 128:(sl + 1) * 128],
                                wv_sb[:, k, :],
                                start=(k == 0), stop=(k == KT - 1))
                        pv_v = pv[:, :].rearrange("p (s h d) -> p s h d",
                                                  s=2, h=HPC)
                        bv_h = bv_bc[:, :].rearrange("p (h d) -> p h d",
                                                     h=HPC)
                        nc.vector.tensor_add(
                            V4[j][:, 2 * sp:2 * sp + 2, :, 0:DK],
                            pv_v, _ins_bcast(bv_h, 1, 2))
                    mq.append((900, vunit_a))
                    mq.append((900, vunit_b))

            def push_cproj_chunk(j, normo):
                last = j == NQ - 1
                no = {}

                def nunit(j=j, normo=normo, no=no):
                    # all normalize multiplies for the chunk in one burst;
                    # by pop time the reciprocal chain has long drained
                    for qb in range(4):
                        qs = slice(qb * 128, (qb + 1) * 128)
                        no[qb] = [
                            pa.tile([128, 128], BF16, tag=f"no{p}",
                                    bufs=8, name=f"no{p}_{j}_{qb}")
                            for p in range(2)]
                        for p in range(2):
                            cbx, rbc = normo[p]
                            nc.vector.tensor_mul(
                                no[qb][p][0:64, :], cbx[0][0:64, qs],
                                rbc[0][:, qs])
                            nc.vector.tensor_mul(
                                no[qb][p][64:128, :], cbx[1][0:64, qs],
                                rbc[1][:, qs])
                oq.append((step_no[0] + 8, 300, nunit))
                ps_t = {}
                for qb in range(4):
                    for nh in range(2):
                        def cunit(j=j, qb=qb, nh=nh, last=last, no=no,
                                  ps_t=ps_t):
                            pt = psum.tile([128, 512], F32, tag="pf", bufs=2,
                                           name=f"pt{j}_{qb}_{nh}")
                            for p in range(2):
                                nc.tensor.matmul(
                                    pt,
                                    no[qb][p][:, :],
                                    wp_sb[:, p, nh * 512:(nh + 1) * 512],
                                    start=(p == 0), stop=(p == 1))
                            ps = pa.tile([128, 512], BF16, tag="ps", bufs=9,
                                         name=f"ps{j}_{qb}_{nh}")
                            if last:
                                nc.scalar.copy(ps, pt)
                            else:
                                nc.vector.tensor_copy(ps, pt)
                            ps_t[(qb, nh)] = ps
                        oq.append((step_no[0] + 9, 500, cunit))

                def cdunit(j=j, ps_t=ps_t):
                    # all 8 partial DMAs in one burst AFTER the evacs have
                    # completed: the sync-queue triggers never block, and
                    # the RS trigger waits only on the DMA semaphores
                    for qb in range(4):
                        for nh in range(2):
                            nc.sync.dma_start(
                                out=partial[j][qb * 128:(qb + 1) * 128,
                                               nh * 512:(nh + 1) * 512],
                                in_=ps_t.pop((qb, nh)))
                    nc.gpsimd.collective_compute(
                        "ReduceScatter", mybir.AluOpType.add,
                        replica_groups=GROUPS,
                        ins=[partial[j].opt()],
                        outs=[rsout[j].opt()])
                    nc.sync.dma_start(out=outs[j][:, :], in_=rsout[j][:, :])
                oq.append((step_no[0] + 17, 300, cdunit))

            # ---------------- attention step machinery ----------------
            normo_h = {}   # (j, pair) -> [even-head tile rows 0-63, odd 64-127]
            pav_h = {}     # (j, pair) -> (pav_e, pav_o)

            def emit_scores(j, pair, ki):
                jk, kb = divmod(ki, 4)
                off = max(0, 128 * ki - CK * j)
                npp = CK - off
                sg = psum.tile([128, 2, CK], F32, tag="sg", bufs=2,
                               name=f"sg{j}_{pair}_{ki}")
                for half in range(2):
                    base = 64 * half
                    nc.tensor.matmul(
                        sg[:, half, off:CK],
                        KTt[jk][base:base + DK, pair,
                                kb * 128:(kb + 1) * 128],
                        QTt[j][base:base + DK, pair, off:CK],
                        start=True, stop=True)
                ag = pa.tile([128, 2, CK], BF16, tag="A", bufs=4,
                             name=f"A{j}_{pair}_{ki}")
                nc.scalar.activation(
                    ag[:, :, off:CK], sg[:, :, off:CK],
                    mybir.ActivationFunctionType.Exp)
                if ki >= 4 * j:  # diagonal block: causal corner mask
                    av = ag[:, :, off:off + 128]
                    nc.vector.tensor_mul(av, av, _ins_bcast(tri[:, :], 1, 2))
                if DEBUG and (j, pair, ki) == (0, 0, 0):
                    nc.sync.dma_start(
                        out=dbg["dbg_ag"].ap().rearrange(
                            "p (h c) -> p h c", h=2),
                        in_=ag)
                return (j, pair, ki, ag, off)

            def emit_av(st):
                j, pair, ki, ag, off = st
                nblk = 4 * j + 4
                if ki == 0:
                    pav_h[(j, pair)] = tuple(
                        psum.tile([65, 512], F32, tag="pav", bufs=2,
                                  name=f"pav{j}_{pair}_{h}")
                        for h in range(2))
                jk, kb = divmod(ki, 4)
                for half in range(2):
                    nc.tensor.matmul(
                        pav_h[(j, pair)][half][:, off:CK],
                        V4[jk][:, kb, 2 * pair + half, :],
                        ag[:, half, off:CK],
                        start=(ki == 0), stop=(ki == nblk - 1))
                if ki == nblk - 1:
                    emit_norm(j, pair)

            def emit_norm(j, pair):
                pav_e, pav_o = pav_h.pop((j, pair))
                # [0:65] copies carry the denominator row along with O^T,
                # releasing the pav PSUM banks in two DVE ops.
                cbx = [pa.tile([65, CK], BF16, tag=f"cbx{h}", bufs=2,
                               name=f"cbx{h}_{j}_{pair}") for h in range(2)]
                cpy = nc.scalar.copy if j == NQ - 1 else nc.vector.tensor_copy
                cpy(cbx[0][0:65, :], pav_e[0:65, :])
                cpy(cbx[1][0:65, :], pav_o[0:65, :])
                srec = pa.tile([128, 2, 4], BF16, tag="srec", bufs=2,
                               name=f"srec{j}_{pair}")
                nc.sync.dma_start(out=srec[:, 0, :], in_=cbx[0][64:65, :])
                nc.sync.dma_start(out=srec[:, 1, :], in_=cbx[1][64:65, :])
                with nc.allow_low_precision(
                        reason="bf16 recip of O(1e2) softmax sums"):
                    nc.vector.reciprocal(srec, srec)
                srd = rsumd[(j * 2 + pair) % 4]
                nc.sync.dma_start(out=srd[0:1, :], in_=srec[:, 0, :])
                nc.sync.dma_start(out=srd[1:2, :], in_=srec[:, 1, :])
                rbc = [pa.tile([64, CK], BF16, tag=f"rbc{h}", bufs=2,
                               name=f"rbc{h}_{j}_{pair}") for h in range(2)]
                nc.gpsimd.dma_start(out=rbc[0][0:64, :],
                                    in_=_bcast_rows(srd[0, :], 64))
                nc.gpsimd.dma_start(out=rbc[1][0:64, :],
                                    in_=_bcast_rows(srd[1, :], 64))
                # the normalize multiplies happen lazily inside the c_proj
                # units, long after this chain has drained
                normo_h[(j, pair)] = (cbx, rbc)
                if DEBUG and j == 0 and pair == 0:
                    nc.sync.dma_start(out=dbg["dbg_comb"][0:64, :],
                                      in_=cbx[0][0:64, :])
                    nc.sync.dma_start(out=dbg["dbg_comb"][64:128, :],
                                      in_=cbx[1][0:64, :])
                    nc.sync.dma_start(out=dbg["dbg_srow"][0:1, :],
                                      in_=cbx[0][64:65, :])
                    nc.sync.dma_start(out=dbg["dbg_srow"][1:2, :],
                                      in_=cbx[1][64:65, :])
                    nc.sync.dma_start(out=dbg["dbg_rbc"][0:64, :], in_=rbc[0])
                    nc.sync.dma_start(out=dbg["dbg_rbc"][64:128, :],
                                      in_=rbc[1])
                    nc.sync.dma_start(out=dbg["dbg_no"][:, :], in_=no)
                if DEBUG and j == 2 and pair == 0:
                    nc.sync.dma_start(out=dbg["dbg_no2"][:, :], in_=no)
                if pair == 1:
                    push_cproj_chunk(j, [normo_h.pop((j, 0)),
                                         normo_h.pop((j, 1))])

            # ---------------- the fused stream ----------------
            steps = [(j, pair, ki)
                     for j in range(NQ)
                     for pair in range(2)
                     for ki in range(4 * j + 4)]

            push_transpose_chunk(0)
            push_qkv_chunk(0)

            pending = deque()   # scores emitted, AV not yet (depth 2)
            cur_chunk = [-1]
            step_no = [0]
            mquota = [0.0]
            mbudget = [0.0]

            for (j, pair, ki) in steps:
                if j != cur_chunk[0]:
                    cur_chunk[0] = j
                    # the current chunk's x^T/QKV must be fully emitted
                    # before its first scores instruction
                    while mq:
                        cost, fn = mq.popleft()
                        fn()
                    if j + 1 < NQ:
                        push_transpose_chunk(j + 1)
                        push_qkv_chunk(j + 1)
                    # ration the next chunk's mandatory work over this
                    # chunk's steps (front-loaded slightly: x0.75 steps)
                    nsteps = 2 * (4 * j + 4)
                    mquota[0] = sum(u[0] for u in mq) / max(nsteps * 0.75, 1)
                    mbudget[0] = 0.0
                pending.append(emit_scores(j, pair, ki))
                mbudget[0] += mquota[0]
                while mq and mbudget[0] >= mq[0][0] * 0.5:
                    cost, fn = mq.popleft()
                    mbudget[0] -= cost
                    fn()
                step_no[0] += 1
                if oq and step_no[0] >= oq[0][0]:
                    # trickle one c_proj unit per step, delayed so the
                    # norm chain has completed by the first pop
                    _, cost, fn = oq.popleft()
                    fn()
                if len(pending) > 2:
                    emit_av(pending.popleft())

            while pending:
                emit_av(pending.popleft())
            while mq:
                cost, fn = mq.popleft()
                fn()
            while oq:
                _, cost, fn = oq.popleft()
                fn()

            if DEBUG:
                nc.sync.dma_start(
                    out=dbg["dbg_qt"].ap().rearrange("p (h c) -> p h c", h=2),
                    in_=QTt[0])
                nc.sync.dma_start(
                    out=dbg["dbg_kt"].ap().rearrange("p (h c) -> p h c", h=2),
                    in_=KTt[0])
                nc.sync.dma_start(
                    out=dbg["dbg_v4"].ap().rearrange(
                        "p (s h d) -> p s h d", s=4, h=HPC),
                    in_=V4[0])
                nc.sync.dma_start(out=dbg["dbg_partial"][:, :],
                                  in_=partial[0][:, :])

    nc.compile()
    return nc


def make_in_maps(x, w_attn, b_attn, w_proj):
    import ml_dtypes
    bf = ml_dtypes.bfloat16
    x = np.asarray(x, dtype=np.float32)
    w_attn = np.asarray(w_attn, dtype=np.float32)
    b_attn = np.asarray(b_attn, dtype=np.float32)
    w_proj = np.asarray(w_proj, dtype=np.float32)
    in_maps = []
    for c in range(NCORES):
        b, g = divmod(c, 4)
        h0 = g * HPC
        cs = slice(h0 * DK, h0 * DK + CW)
        wpc = np.ascontiguousarray(w_proj[cs, :])          # [256, H]
        wp_pair = np.ascontiguousarray(
            wpc.reshape(2, 128, H).transpose(1, 0, 2).reshape(128, 2 * H))
        in_maps.append({
            "xb": np.ascontiguousarray(x[b]).astype(bf),
            # fold the 1/sqrt(DK)=2^-3 score scale into Wq/bq (exact in fp32)
            "wq": (np.ascontiguousarray(w_attn[:, cs])
                   * np.float32(0.125)).astype(bf),
            "wk": np.ascontiguousarray(w_attn[:, H:][:, cs]).astype(bf),
            "wv": np.ascontiguousarray(w_attn[:, 2 * H:][:, cs]).astype(bf),
            "wp": wp_pair.astype(bf),
            "bq": np.ascontiguousarray(b_attn[cs]) * np.float32(0.125),
            "bk": np.ascontiguousarray(b_attn[H:][cs]),
            "bv": np.ascontiguousarray(b_attn[2 * H:][cs]),
        })
    return in_maps


_NC = None


def kernel(x, w_attn, b_attn, w_proj, b_proj):
    global _NC
    if _NC is None:
        _NC = build_nc()

    b_proj = np.asarray(b_proj, dtype=np.float32)
    in_maps = make_in_maps(x, w_attn, b_attn, w_proj)
    res = run_bass_kernel_spmd(_NC, in_maps, core_ids=list(range(NCORES)))

    outp = np.empty((B, S, H), dtype=np.float32)
    for c in range(NCORES):
        b, g = divmod(c, 4)
        for j in range(NQ):
            o = res.results[c][f"out{j}"].astype(np.float32)
            outp[b, j * CK + g * 128: j * CK + (g + 1) * 128, :] = o
    outp += b_proj  # row-broadcast add, exact
    return outp


# revision 44
# speedup vs baseline: 1.2505x; 1.1521x over previous
"""Causal self-attention (B=2, S=2048, H=1024, NH=16) on 8 TRN2 NeuronCores.

Sharding: core c handles batch b = c//4 and heads [4*(c%4), 4*(c%4)+4).
Tensor-parallel c_attn (column split) AND tensor-parallel c_proj (row
split over this core's 4 head-dims), with a per-chunk bf16 ReduceScatter
over the 4-core batch group producing each core's final output rows.
No gather of O^T and no data-driven offsets are needed.

Single fused software-pipelined stream per core:
  - Attention runs as a flat sequence of (pair, key-block) steps,
    j-chunk outer.  A "pair" is two heads whose Q^T/K^T live on SBUF
    partitions 0-63 (even head) and 64-127 (odd head): their score
    matmuls are issued back-to-back so the PE's 64-row tiling
    (auto-derived tile_position h0/h64) executes them concurrently.
  - exp runs on ACT once per (pair, block) over both heads
    [128, 2, npp] straight out of PSUM (ACT is the critical engine:
    ~80us of exp); scores are issued 2 steps ahead of the matching
    A^T@V so the PE never waits on ACT.
  - The next chunk's x^T transposes (matmul-form: x-block stationary,
    identity streaming - unlike transpose-mode these engage the HAM
    clock un-throttle) and QKV matmuls, and the previous chunk's
    c_proj partial matmuls, are interleaved between attention steps
    as rationed "filler" so the PE stays dense and warm.
  - Softmax normalization per (chunk, pair): ones-column denominators
    from PSUM row 64, PSUM->SBUF reshape DMA, one batched DVE
    reciprocal, one stride-0 broadcast DMA per head, one DVE multiply
    into the pair-packed normalized O^T tile that c_proj consumes.
  - c_proj contracts the core's own 256 head-dims (full-128 matmuls
    over head pairs) per 128-query block; partials go to DRAM in bf16
    and a per-chunk ReduceScatter(add) over the batch group yields the
    core's 128-row slice of each chunk.

All matmul operands bf16 (Wq pre-scaled by 1/8), fp32 PSUM.  b_proj is
added on the host (exact), QKV biases on DVE/GPSIMD during PSUM
evacuation.
"""

import sys

sys.path.insert(0, "/opt/trn_rl_repo")

from collections import deque

import numpy as np

import concourse.bass as bass
import concourse.mybir as mybir
import concourse.tile as tile
from concourse import bacc
from concourse.bass_utils import run_bass_kernel_spmd
from concourse.masks import make_identity

B, S, H, NH, DK = 2, 2048, 1024, 16, 64
NCORES = 8
HPC = 4            # heads per core
CW = HPC * DK      # 256 qkv columns per core
GROUPS = [[0, 1, 2, 3], [4, 5, 6, 7]]

F32 = mybir.dt.float32
BF16 = mybir.dt.bfloat16

DEBUG = False

KT = H // 128   # 8 contraction tiles over H
NQ = S // 512   # 4 query chunks of 512
CK = 512        # chunk width


def _ins_bcast(ap, pos, n):
    """Insert a stride-0 (broadcast) dim of size n at free-dim position pos."""
    a = [list(p) for p in ap.ap]
    return bass.AP(tensor=ap.tensor, offset=ap.offset,
                   ap=a[:pos] + [[0, n]] + a[pos:])


def _bcast_rows(src_ap, parts):
    """Stride-0 partition broadcast: repeat src_ap's whole AP across parts."""
    a = [list(p) for p in src_ap.ap]
    return bass.AP(tensor=src_ap.tensor, offset=src_ap.offset,
                   ap=[[0, parts]] + a)


def build_nc():
    nc = bacc.Bacc(None, target_bir_lowering=False, debug=False,
                   num_devices=NCORES)

    xb = nc.declare_dram_parameter("xb", [S, H], BF16, isOutput=False)
    wq = nc.declare_dram_parameter("wq", [H, CW], BF16, isOutput=False)
    wk = nc.declare_dram_parameter("wk", [H, CW], BF16, isOutput=False)
    wv = nc.declare_dram_parameter("wv", [H, CW], BF16, isOutput=False)
    # c_proj rows for this core's 4 heads, pair-packed:
    # wp[d, p*H + n] = w_proj[h0*64 + 128*p + d, n]
    wp = nc.declare_dram_parameter("wp", [128, 2 * H], BF16, isOutput=False)
    bq = nc.declare_dram_parameter("bq", [CW], F32, isOutput=False)
    bk = nc.declare_dram_parameter("bk", [CW], F32, isOutput=False)
    bv = nc.declare_dram_parameter("bv", [CW], F32, isOutput=False)
    outs = [nc.declare_dram_parameter(f"out{j}", [128, H], BF16, isOutput=True)
            for j in range(NQ)]
    dbg = {}
    if DEBUG:
        for nm, shape, dt in [
            ("dbg_qt", [128, 2 * CK], BF16), ("dbg_kt", [128, 2 * CK], BF16),
            ("dbg_v4", [128, 4 * HPC * (DK + 1)], BF16),
            ("dbg_comb", [128, CK], BF16), ("dbg_srow", [2, CK], BF16),
            ("dbg_rbc", [128, CK], BF16), ("dbg_no", [128, CK], BF16),
            ("dbg_no2", [128, CK], BF16), ("dbg_partial", [CK, H], BF16),
            ("dbg_ag", [128, 2 * CK], BF16),
        ]:
            dbg[nm] = nc.declare_dram_parameter(nm, shape, dt, isOutput=True)

    with tile.TileContext(nc) as tc:
        with (
            tc.tile_pool(name="dram", bufs=1, space="DRAM") as dram,
            tc.tile_pool(name="psum", bufs=1, space="PSUM") as psum,
            tc.tile_pool(name="persist", bufs=1) as pw,
            tc.tile_pool(name="work", bufs=1) as pa,
        ):
            partial = [dram.tile([CK, H], BF16, name=f"partial{j}")
                       for j in range(NQ)]
            rsout = [dram.tile([128, H], BF16, name=f"rsout{j}")
                     for j in range(NQ)]
            # DRAM bounce rows for the reciprocal broadcast (stride-0
            # partition-source DMA is only legal from DRAM)
            rsumd = [dram.tile([2, CK], BF16, name=f"rsumd{i}")
                     for i in range(4)]
            # contiguous per-column-half tiles for the last chunk's split RS
            partial3h = [dram.tile([CK, 512], BF16, name=f"partial3h{h}")
                         for h in range(2)]
            rsout3h = [dram.tile([128, 512], BF16, name=f"rsout3h{h}")
                       for h in range(2)]

            ident = pw.tile([128, 128], BF16)
            ident_f32 = pw.tile([128, 128], F32)
            make_identity(nc, ident_f32)
            nc.vector.tensor_copy(ident, ident_f32)
            # lower-triangle-in-q mask: tri[k, q] = 1 if q >= k else 0
            tri_f32 = pw.tile([128, 128], F32)
            nc.gpsimd.memset(tri_f32, 1.0)
            nc.gpsimd.affine_select(
                out=tri_f32, in_=tri_f32, compare_op=mybir.AluOpType.is_ge,
                fill=0.0, base=0, pattern=[[1, 128]], channel_multiplier=-1)
            tri = pw.tile([128, 128], BF16)
            nc.vector.tensor_copy(tri, tri_f32)

            # weights: [128, k-tile, cols]
            wq_sb = pw.tile([128, KT, CW], BF16)
            wk_sb = pw.tile([128, KT, CW], BF16)
            wv_sb = pw.tile([128, KT, CW], BF16)
            nc.gpsimd.dma_start(
                out=wq_sb, in_=wq.ap().rearrange("(k p) c -> p k c", p=128))
            nc.gpsimd.dma_start(
                out=wk_sb, in_=wk.ap().rearrange("(k p) c -> p k c", p=128))
            nc.gpsimd.dma_start(
                out=wv_sb, in_=wv.ap().rearrange("(k p) c -> p k c", p=128))
            wp_sb = pw.tile([128, 2, H], BF16)
            nc.gpsimd.dma_start(
                out=wp_sb, in_=wp.ap().rearrange("p (r c) -> p r c", r=2))

            bq_sb = pw.tile([128, 2], F32)
            bk_sb = pw.tile([128, 2], F32)
            nc.gpsimd.dma_start(out=bq_sb,
                                in_=bq.ap().rearrange("(h p) -> p h", p=128))
            nc.gpsimd.dma_start(out=bk_sb,
                                in_=bk.ap().rearrange("(h p) -> p h", p=128))
            bv_bc = pw.tile([128, CW], F32)
            nc.gpsimd.dma_start(out=bv_bc, in_=_bcast_rows(bv.ap(), 128))

            # tiny warmup collective: absorbs the ~11us first-collective
            # trigger delay during the prologue
            warm_in = dram.tile([4, H], BF16, name="warm_in")
            warm_out = dram.tile([1, H], BF16, name="warm_out")
            nc.gpsimd.collective_compute(
                "ReduceScatter", mybir.AluOpType.add,
                replica_groups=GROUPS,
                ins=[warm_in.opt()], outs=[warm_out.opt()])

            # per-chunk tensors (separate tiles -> no false dependencies)
            xT = [pw.tile([128, KT, CK], BF16, name=f"xT{j}")
                  for j in range(NQ)]
            QTt = [pw.tile([128, 2, CK], BF16, name=f"QT{j}")
                   for j in range(NQ)]
            KTt = [pw.tile([128, 2, CK], BF16, name=f"KT{j}")
                   for j in range(NQ)]
            V4 = [pw.tile([128, 4, HPC, DK + 1], BF16, name=f"V4{j}")
                  for j in range(NQ)]
            for j in range(NQ):
                nc.gpsimd.memset(V4[j], 1.0)  # ones col (rest overwritten)

            # ---------------- filler unit machinery ----------------
            # Each unit: (kind, chunk, cost_ns, closure).  kinds 't'/'q'
            # must be flushed before the first attention step of their
            # chunk; 'c' (c_proj) units are flexible.
            mq = deque()
            oq = deque()
            def tr_evac(dst, src):
                nc.vector.tensor_copy(dst, src)

            def push_transpose_chunk(j):
                """x^T for chunk j: 4 xs DMAs now, 8 matmul-form units."""
                xs_tiles = []
                for sl in range(4):
                    si = 4 * j + sl
                    xs = pa.tile([128, H], BF16, tag="xs", bufs=4,
                                 name=f"xs{si}")
                    nc.sync.dma_start(out=xs,
                                      in_=xb[si * 128:(si + 1) * 128, :])
                    xs_tiles.append(xs)
                for sl in range(4):
                    for kh in range(2):
                        def tunit(j=j, sl=sl, kh=kh, xs=xs_tiles[sl]):
                            ptp = psum.tile([128, 512], F32, tag="pf", bufs=2,
                                            name=f"ptp{j}_{sl}_{kh}")
                            for i in range(4):
                                k = 4 * kh + i
                                nc.tensor.matmul(
                                    ptp[:, i * 128:(i + 1) * 128],
                                    xs[:, k * 128:(k + 1) * 128],
                                    ident, start=True, stop=True)
                            src = ptp[:, :].rearrange("p (k f) -> p k f", k=4)
                            dst = xT[j][:, 4 * kh:4 * kh + 4,
                                        sl * 128:(sl + 1) * 128]
                            tr_evac(dst, src)
                        mq.append((350, tunit))

            def push_qkv_chunk(j):
                for ti, (wt, dst, bias) in enumerate(
                        ((wq_sb, QTt[j], bq_sb), (wk_sb, KTt[j], bk_sb))):
                    for half in range(2):
                        pqs = {}

                        def qunit_a(j=j, wt=wt, half=half, pqs=pqs, ti=ti):
                            pq = psum.tile([128, 512], F32, tag="pf", bufs=2,
                                           name=f"pq{j}_{ti}_{half}")
                            pqs['t'] = pq
                            for k in range(4):
                                nc.tensor.matmul(
                                    pq,
                                    wt[:, k, half * 128:(half + 1) * 128],
                                    xT[j][:, k, :],
                                    start=(k == 0), stop=False)

                        def qunit_b(j=j, ti=ti, wt=wt, dst=dst, bias=bias,
                                    half=half, pqs=pqs):
                            pq = pqs.pop('t')
                            for k in range(4, KT):
                                nc.tensor.matmul(
                                    pq,
                                    wt[:, k, half * 128:(half + 1) * 128],
                                    xT[j][:, k, :],
                                    start=False, stop=(k == KT - 1))
                            if ti == 0:
                                nc.scalar.activation(
                                    dst[:, half, :], pq,
                                    mybir.ActivationFunctionType.Identity,
                                    bias=bias[:, half:half + 1])
                            else:
                                nc.vector.tensor_scalar_add(
                                    dst[:, half, :], pq,
                                    bias[:, half:half + 1])
                        mq.append((900, qunit_a))
                        mq.append((900, qunit_b))
                for sp in range(2):  # si pairs (0,1), (2,3)
                    pvs = {}

                    def vunit_a(j=j, sp=sp, pvs=pvs):
                        pv = psum.tile([128, 512], F32, tag="pf", bufs=2,
                                       name=f"pv{j}_{sp}")
                        pvs['t'] = pv
                        sl = 2 * sp
                        for k in range(KT):
                            nc.tensor.matmul(
                                pv[:, 0:256],
                                xT[j][:, k, sl * 128:(sl + 1) * 128],
                                wv_sb[:, k, :],
                                start=(k == 0), stop=(k == KT - 1))

                    def vunit_b(j=j, sp=sp, pvs=pvs):
                        pv = pvs.pop('t')
                        sl = 2 * sp + 1
                        for k in range(KT):
                            nc.tensor.matmul(
                                pv[:, 256:512],
                                xT[j][:, k, sl * 128:(sl + 1) * 128],
                                wv_sb[:, k, :],
                                start=(k == 0), stop=(k == KT - 1))
                        pv_v = pv[:, :].rearrange("p (s h d) -> p s h d",
                                                  s=2, h=HPC)
                        bv_h = bv_bc[:, :].rearrange("p (h d) -> p h d",
                                                     h=HPC)
                        nc.vector.tensor_add(
                            V4[j][:, 2 * sp:2 * sp + 2, :, 0:DK],
                            pv_v, _ins_bcast(bv_h, 1, 2))
                    mq.append((900, vunit_a))
                    mq.append((900, vunit_b))

            def push_cproj_chunk(j, normo):
                last = j == NQ - 1
                no = {}

                def nunit(j=j, normo=normo, no=no):
                    # all normalize multiplies for the chunk in one burst;
                    # by pop time the reciprocal chain has long drained
                    for qb in range(4):
                        qs = slice(qb * 128, (qb + 1) * 128)
                        no[qb] = [
                            pa.tile([128, 128], BF16, tag=f"no{p}",
                                    bufs=8, name=f"no{p}_{j}_{qb}")
                            for p in range(2)]
                        for p in range(2):
                            cbx, rbc = normo[p]
                            nc.vector.tensor_mul(
                                no[qb][p][0:64, :], cbx[0][0:64, qs],
                                rbc[0][:, qs])
                            nc.vector.tensor_mul(
                                no[qb][p][64:128, :], cbx[1][0:64, qs],
                                rbc[1][:, qs])
                oq.append((step_no[0] + 8, 300, nunit))
                ps_t = {}
                for qb in range(4):
                    for nh in range(2):
                        def cunit(j=j, qb=qb, nh=nh, last=last, no=no,
                                  ps_t=ps_t):
                            pt = psum.tile([128, 512], F32, tag="pf", bufs=2,
                                           name=f"pt{j}_{qb}_{nh}")
                            for p in range(2):
                                nc.tensor.matmul(
                                    pt,
                                    no[qb][p][:, :],
                                    wp_sb[:, p, nh * 512:(nh + 1) * 512],
                                    start=(p == 0), stop=(p == 1))
                            ps = pa.tile([128, 512], BF16, tag="ps", bufs=9,
                                         name=f"ps{j}_{qb}_{nh}")
                            if last:
                                nc.scalar.copy(ps, pt)
                            else:
                                nc.vector.tensor_copy(ps, pt)
                            ps_t[(qb, nh)] = ps
                        oq.append((step_no[0] + 9, 500, cunit))

                def cdunit(j=j, ps_t=ps_t):
                    # all 8 partial DMAs in one burst AFTER the evacs have
                    # completed: the sync-queue triggers never block, and
                    # the RS trigger waits only on the DMA semaphores
                    for qb in range(4):
                        for nh in range(2):
                            nc.sync.dma_start(
                                out=partial[j][qb * 128:(qb + 1) * 128,
                                               nh * 512:(nh + 1) * 512],
                                in_=ps_t.pop((qb, nh)))
                    nc.gpsimd.collective_compute(
                        "ReduceScatter", mybir.AluOpType.add,
                        replica_groups=GROUPS,
                        ins=[partial[j].opt()],
                        outs=[rsout[j].opt()])
                    nc.sync.dma_start(out=outs[j][:, :], in_=rsout[j][:, :])
                oq.append((step_no[0] + 17, 300, cdunit))

            # ---------------- attention step machinery ----------------
            normo_h = {}   # (j, pair) -> [even-head tile rows 0-63, odd 64-127]
            pav_h = {}     # (j, pair) -> (pav_e, pav_o)

            def emit_scores(j, pair, ki):
                jk, kb = divmod(ki, 4)
                off = max(0, 128 * ki - CK * j)
                npp = CK - off
                sg = psum.tile([128, 2, CK], F32, tag="sg", bufs=2,
                               name=f"sg{j}_{pair}_{ki}")
                for half in range(2):
                    base = 64 * half
                    nc.tensor.matmul(
                        sg[:, half, off:CK],
                        KTt[jk][base:base + DK, pair,
                                kb * 128:(kb + 1) * 128],
                        QTt[j][base:base + DK, pair, off:CK],
                        start=True, stop=True)
                ag = pa.tile([128, 2, CK], BF16, tag="A", bufs=4,
                             name=f"A{j}_{pair}_{ki}")
                nc.scalar.activation(
                    ag[:, :, off:CK], sg[:, :, off:CK],
                    mybir.ActivationFunctionType.Exp)
                if ki >= 4 * j:  # diagonal block: causal corner mask
                    av = ag[:, :, off:off + 128]
                    nc.vector.tensor_mul(av, av, _ins_bcast(tri[:, :], 1, 2))
                if DEBUG and (j, pair, ki) == (0, 0, 0):
                    nc.sync.dma_start(
                        out=dbg["dbg_ag"].ap().rearrange(
                            "p (h c) -> p h c", h=2),
                        in_=ag)
                return (j, pair, ki, ag, off)

            def emit_av(st):
                j, pair, ki, ag, off = st
                nblk = 4 * j + 4
                if ki == 0:
                    pav_h[(j, pair)] = tuple(
                        psum.tile([65, 512], F32, tag="pav", bufs=2,
                                  name=f"pav{j}_{pair}_{h}")
                        for h in range(2))
                jk, kb = divmod(ki, 4)
                for half in range(2):
                    nc.tensor.matmul(
                        pav_h[(j, pair)][half][:, off:CK],
                        V4[jk][:, kb, 2 * pair + half, :],
                        ag[:, half, off:CK],
                        start=(ki == 0), stop=(ki == nblk - 1))
                if ki == nblk - 1:
                    emit_norm(j, pair)

            def emit_norm(j, pair):
                pav_e, pav_o = pav_h.pop((j, pair))
                # [0:65] copies carry the denominator row along with O^T,
                # releasing the pav PSUM banks in two DVE ops.
                cbx = [pa.tile([65, CK], BF16, tag=f"cbx{h}", bufs=2,
                               name=f"cbx{h}_{j}_{pair}") for h in range(2)]
                cpy = nc.scalar.copy if j == NQ - 1 else nc.vector.tensor_copy
                cpy(cbx[0][0:65, :], pav_e[0:65, :])
                cpy(cbx[1][0:65, :], pav_o[0:65, :])
                srec = pa.tile([128, 2, 4], BF16, tag="srec", bufs=2,
                               name=f"srec{j}_{pair}")
                nc.gpsimd.dma_start(out=srec[:, 0, :], in_=cbx[0][64:65, :])
                nc.gpsimd.dma_start(out=srec[:, 1, :], in_=cbx[1][64:65, :])
                with nc.allow_low_precision(
                        reason="bf16 recip of O(1e2) softmax sums"):
                    nc.vector.reciprocal(srec, srec)
                srr = [pa.tile([1, CK], BF16, tag=f"srr{h}", bufs=2,
                               name=f"srr{h}_{j}_{pair}") for h in range(2)]
                nc.gpsimd.dma_start(out=srr[0][0:1, :], in_=srec[:, 0, :])
                nc.gpsimd.dma_start(out=srr[1][0:1, :], in_=srec[:, 1, :])
                rbc = [pa.tile([64, CK], BF16, tag=f"rbc{h}", bufs=2,
                               name=f"rbc{h}_{j}_{pair}") for h in range(2)]
                nc.gpsimd.partition_broadcast(rbc[0][0:64, :], srr[0][0:1, :],
                                              channels=64)
                nc.gpsimd.partition_broadcast(rbc[1][0:64, :], srr[1][0:1, :],
                                              channels=64)
                # the normalize multiplies happen lazily inside the c_proj
                # units, long after this chain has drained
                normo_h[(j, pair)] = (cbx, rbc)
                if DEBUG and j == 0 and pair == 0:
                    nc.sync.dma_start(out=dbg["dbg_comb"][0:64, :],
                                      in_=cbx[0][0:64, :])
                    nc.sync.dma_start(out=dbg["dbg_comb"][64:128, :],
                                      in_=cbx[1][0:64, :])
                    nc.sync.dma_start(out=dbg["dbg_srow"][0:1, :],
                                      in_=cbx[0][64:65, :])
                    nc.sync.dma_start(out=dbg["dbg_srow"][1:2, :],
                                      in_=cbx[1][64:65, :])
                    nc.sync.dma_start(out=dbg["dbg_rbc"][0:64, :], in_=rbc[0])
                    nc.sync.dma_start(out=dbg["dbg_rbc"][64:128, :],
                                      in_=rbc[1])
                    nc.sync.dma_start(out=dbg["dbg_no"][:, :], in_=no)
                if DEBUG and j == 2 and pair == 0:
                    nc.sync.dma_start(out=dbg["dbg_no2"][:, :], in_=no)
                if pair == 1:
                    push_cproj_chunk(j, [normo_h.pop((j, 0)),
                                         normo_h.pop((j, 1))])

            # ---------------- the fused stream ----------------
            steps = [(j, pair, ki)
                     for j in range(NQ)
                     for pair in range(2)
                     for ki in range(4 * j + 4)]

            push_transpose_chunk(0)
            push_qkv_chunk(0)

            pending = deque()   # scores emitted, AV not yet (depth 2)
            cur_chunk = [-1]
            step_no = [0]
            mquota = [0.0]
            mbudget = [0.0]

            for (j, pair, ki) in steps:
                if j != cur_chunk[0]:
                    cur_chunk[0] = j
                    # the current chunk's x^T/QKV must be fully emitted
                    # before its first scores instruction
                    while mq:
                        cost, fn = mq.popleft()
                        fn()
                    if j + 1 < NQ:
                        push_transpose_chunk(j + 1)
                        push_qkv_chunk(j + 1)
                    # ration the next chunk's mandatory work over this
                    # chunk's steps (front-loaded slightly: x0.75 steps)
                    nsteps = 2 * (4 * j + 4)
                    mquota[0] = sum(u[0] for u in mq) / max(nsteps * 0.75, 1)
                    mbudget[0] = 0.0
                pending.append(emit_scores(j, pair, ki))
                mbudget[0] += mquota[0]
                while mq and mbudget[0] >= mq[0][0] * 0.5:
                    cost, fn = mq.popleft()
                    mbudget[0] -= cost
                    fn()
                step_no[0] += 1
                if oq and step_no[0] >= oq[0][0]:
                    # trickle one c_proj unit per step, delayed so the
                    # norm chain has completed by the first pop
                    _, cost, fn = oq.popleft()
                    fn()
                if len(pending) > 2:
                    emit_av(pending.popleft())

            while pending:
                emit_av(pending.popleft())
            while mq:
                cost, fn = mq.popleft()
                fn()
            while oq:
                _, cost, fn = oq.popleft()
                fn()

            if DEBUG:
                nc.sync.dma_start(
                    out=dbg["dbg_qt"].ap().rearrange("p (h c) -> p h c", h=2),
                    in_=QTt[0])
                nc.sync.dma_start(
                    out=dbg["dbg_kt"].ap().rearrange("p (h c) -> p h c", h=2),
                    in_=KTt[0])
                nc.sync.dma_start(
                    out=dbg["dbg_v4"].ap().rearrange(
                        "p (s h d) -> p s h d", s=4, h=HPC),
                    in_=V4[0])
                nc.sync.dma_start(out=dbg["dbg_partial"][:, :],
                                  in_=partial[0][:, :])

    nc.compile()
    return nc


def make_in_maps(x, w_attn, b_attn, w_proj):
    import ml_dtypes
    bf = ml_dtypes.bfloat16
    x = np.asarray(x, dtype=np.float32)
    w_attn = np.asarray(w_attn, dtype=np.float32)
    b_attn = np.asarray(b_attn, dtype=np.float32)
    w_proj = np.asarray(w_proj, dtype=np.float32)
    in_maps = []
    for c in range(NCORES):
        b, g = divmod(c, 4)
        h0 = g * HPC
        cs = slice(h0 * DK, h0 * DK + CW)
        wpc = np.ascontiguousarray(w_proj[cs, :])          # [256, H]
        wp_pair = np.ascontiguousarray(
            wpc.reshape(2, 128, H).transpose(1, 0, 2).reshape(128, 2 * H))
        in_maps.append({
            "xb": np.ascontiguousarray(x[b]).astype(bf),
            # fold the 1/sqrt(DK)=2^-3 score scale into Wq/bq (exact in fp32)
            "wq": (np.ascontiguousarray(w_attn[:, cs])
                   * np.float32(0.125)).astype(bf),
            "wk": np.ascontiguousarray(w_attn[:, H:][:, cs]).astype(bf),
            "wv": np.ascontiguousarray(w_attn[:, 2 * H:][:, cs]).astype(bf),
            "wp": wp_pair.astype(bf),
            "bq": np.ascontiguousarray(b_attn[cs]) * np.float32(0.125),
            "bk": np.ascontiguousarray(b_attn[H:][cs]),
            "bv": np.ascontiguousarray(b_attn[2 * H:][cs]),
        })
    return in_maps


_NC = None


def kernel(x, w_attn, b_attn, w_proj, b_proj):
    global _NC
    if _NC is None:
        _NC = build_nc()

    b_proj = np.asarray(b_proj, dtype=np.float32)
    in_maps = make_in_maps(x, w_attn, b_attn, w_proj)
    res = run_bass_kernel_spmd(_NC, in_maps, core_ids=list(range(NCORES)))

    outp = np.empty((B, S, H), dtype=np.float32)
    for c in range(NCORES):
        b, g = divmod(c, 4)
        for j in range(NQ):
            o = res.results[c][f"out{j}"].astype(np.float32)
            outp[b, j * CK + g * 128: j * CK + (g + 1) * 128, :] = o
    outp += b_proj  # row-broadcast add, exact
    return outp


# revision 46
# speedup vs baseline: 1.2788x; 1.0226x over previous
"""Causal self-attention (B=2, S=2048, H=1024, NH=16) on 8 TRN2 NeuronCores.

Sharding: core c handles batch b = c//4 and heads [4*(c%4), 4*(c%4)+4).
Tensor-parallel c_attn (column split) AND tensor-parallel c_proj (row
split over this core's 4 head-dims), with a per-chunk bf16 ReduceScatter
over the 4-core batch group producing each core's final output rows.
No gather of O^T and no data-driven offsets are needed.

Single fused software-pipelined stream per core:
  - Attention runs as a flat sequence of (pair, key-block) steps,
    j-chunk outer.  A "pair" is two heads whose Q^T/K^T live on SBUF
    partitions 0-63 (even head) and 64-127 (odd head): their score
    matmuls are issued back-to-back so the PE's 64-row tiling
    (auto-derived tile_position h0/h64) executes them concurrently.
  - exp runs on ACT once per (pair, block) over both heads
    [128, 2, npp] straight out of PSUM (ACT is the critical engine:
    ~80us of exp); scores are issued 2 steps ahead of the matching
    A^T@V so the PE never waits on ACT.
  - The next chunk's x^T transposes (matmul-form: x-block stationary,
    identity streaming - unlike transpose-mode these engage the HAM
    clock un-throttle) and QKV matmuls, and the previous chunk's
    c_proj partial matmuls, are interleaved between attention steps
    as rationed "filler" so the PE stays dense and warm.
  - Softmax normalization per (chunk, pair): ones-column denominators
    from PSUM row 64, PSUM->SBUF reshape DMA, one batched DVE
    reciprocal, one stride-0 broadcast DMA per head, one DVE multiply
    into the pair-packed normalized O^T tile that c_proj consumes.
  - c_proj contracts the core's own 256 head-dims (full-128 matmuls
    over head pairs) per 128-query block; partials go to DRAM in bf16
    and a per-chunk ReduceScatter(add) over the batch group yields the
    core's 128-row slice of each chunk.

All matmul operands bf16 (Wq pre-scaled by 1/8), fp32 PSUM.  b_proj is
added on the host (exact), QKV biases on DVE/GPSIMD during PSUM
evacuation.
"""

import sys

sys.path.insert(0, "/opt/trn_rl_repo")

from collections import deque

import numpy as np

import concourse.bass as bass
import concourse.mybir as mybir
import concourse.tile as tile
from concourse import bacc
from concourse.bass_utils import run_bass_kernel_spmd
from concourse.masks import make_identity

B, S, H, NH, DK = 2, 2048, 1024, 16, 64
NCORES = 8
HPC = 4            # heads per core
CW = HPC * DK      # 256 qkv columns per core
GROUPS = [[0, 1, 2, 3], [4, 5, 6, 7]]

F32 = mybir.dt.float32
BF16 = mybir.dt.bfloat16

DEBUG = False

KT = H // 128   # 8 contraction tiles over H
NQ = S // 512   # 4 query chunks of 512
CK = 512        # chunk width


def _ins_bcast(ap, pos, n):
    """Insert a stride-0 (broadcast) dim of size n at free-dim position pos."""
    a = [list(p) for p in ap.ap]
    return bass.AP(tensor=ap.tensor, offset=ap.offset,
                   ap=a[:pos] + [[0, n]] + a[pos:])


def _bcast_rows(src_ap, parts):
    """Stride-0 partition broadcast: repeat src_ap's whole AP across parts."""
    a = [list(p) for p in src_ap.ap]
    return bass.AP(tensor=src_ap.tensor, offset=src_ap.offset,
                   ap=[[0, parts]] + a)


def build_nc():
    nc = bacc.Bacc(None, target_bir_lowering=False, debug=False,
                   num_devices=NCORES)

    xb = nc.declare_dram_parameter("xb", [S, H], BF16, isOutput=False)
    wq = nc.declare_dram_parameter("wq", [H, CW], BF16, isOutput=False)
    wk = nc.declare_dram_parameter("wk", [H, CW], BF16, isOutput=False)
    wv = nc.declare_dram_parameter("wv", [H, CW], BF16, isOutput=False)
    # c_proj rows for this core's 4 heads, pair-packed:
    # wp[d, p*H + n] = w_proj[h0*64 + 128*p + d, n]
    wp = nc.declare_dram_parameter("wp", [128, 2 * H], BF16, isOutput=False)
    bq = nc.declare_dram_parameter("bq", [CW], F32, isOutput=False)
    bk = nc.declare_dram_parameter("bk", [CW], F32, isOutput=False)
    bv = nc.declare_dram_parameter("bv", [CW], F32, isOutput=False)
    outs = [nc.declare_dram_parameter(f"out{j}", [128, H], BF16, isOutput=True)
            for j in range(NQ)]
    dbg = {}
    if DEBUG:
        for nm, shape, dt in [
            ("dbg_qt", [128, 2 * CK], BF16), ("dbg_kt", [128, 2 * CK], BF16),
            ("dbg_v4", [128, 4 * HPC * (DK + 1)], BF16),
            ("dbg_comb", [128, CK], BF16), ("dbg_srow", [2, CK], BF16),
            ("dbg_rbc", [128, CK], BF16), ("dbg_no", [128, CK], BF16),
            ("dbg_no2", [128, CK], BF16), ("dbg_partial", [CK, H], BF16),
            ("dbg_ag", [128, 2 * CK], BF16),
        ]:
            dbg[nm] = nc.declare_dram_parameter(nm, shape, dt, isOutput=True)

    with tile.TileContext(nc) as tc:
        with (
            tc.tile_pool(name="dram", bufs=1, space="DRAM") as dram,
            tc.tile_pool(name="psum", bufs=1, space="PSUM") as psum,
            tc.tile_pool(name="persist", bufs=1) as pw,
            tc.tile_pool(name="work", bufs=1) as pa,
        ):
            partial = [dram.tile([CK, H], BF16, name=f"partial{j}")
                       for j in range(NQ)]
            rsout = [dram.tile([128, H], BF16, name=f"rsout{j}")
                     for j in range(NQ)]
            # DRAM bounce rows for the reciprocal broadcast (stride-0
            # partition-source DMA is only legal from DRAM)
            rsumd = [dram.tile([2, CK], BF16, name=f"rsumd{i}")
                     for i in range(4)]
            # contiguous per-column-half tiles for the last chunk's split RS
            partial3h = [dram.tile([CK, 512], BF16, name=f"partial3h{h}")
                         for h in range(2)]
            rsout3h = [dram.tile([128, 512], BF16, name=f"rsout3h{h}")
                       for h in range(2)]

            ident = pw.tile([128, 128], BF16)
            ident_f32 = pw.tile([128, 128], F32)
            make_identity(nc, ident_f32)
            nc.vector.tensor_copy(ident, ident_f32)
            # lower-triangle-in-q mask: tri[k, q] = 1 if q >= k else 0
            tri_f32 = pw.tile([128, 128], F32)
            nc.gpsimd.memset(tri_f32, 1.0)
            nc.gpsimd.affine_select(
                out=tri_f32, in_=tri_f32, compare_op=mybir.AluOpType.is_ge,
                fill=0.0, base=0, pattern=[[1, 128]], channel_multiplier=-1)
            tri = pw.tile([128, 128], BF16)
            nc.vector.tensor_copy(tri, tri_f32)

            # weights: [128, k-tile, cols]
            wq_sb = pw.tile([128, KT, CW], BF16)
            wk_sb = pw.tile([128, KT, CW], BF16)
            wv_sb = pw.tile([128, KT, CW], BF16)
            nc.gpsimd.dma_start(
                out=wq_sb, in_=wq.ap().rearrange("(k p) c -> p k c", p=128))
            nc.gpsimd.dma_start(
                out=wk_sb, in_=wk.ap().rearrange("(k p) c -> p k c", p=128))
            nc.gpsimd.dma_start(
                out=wv_sb, in_=wv.ap().rearrange("(k p) c -> p k c", p=128))
            wp_sb = pw.tile([128, 2, H], BF16)
            nc.gpsimd.dma_start(
                out=wp_sb, in_=wp.ap().rearrange("p (r c) -> p r c", r=2))

            bq_sb = pw.tile([128, 2], F32)
            bk_sb = pw.tile([128, 2], F32)
            nc.gpsimd.dma_start(out=bq_sb,
                                in_=bq.ap().rearrange("(h p) -> p h", p=128))
            nc.gpsimd.dma_start(out=bk_sb,
                                in_=bk.ap().rearrange("(h p) -> p h", p=128))
            bv_bc = pw.tile([128, CW], F32)
            nc.gpsimd.dma_start(out=bv_bc, in_=_bcast_rows(bv.ap(), 128))

            # tiny warmup collective: absorbs the ~11us first-collective
            # trigger delay during the prologue
            warm_in = dram.tile([4, H], BF16, name="warm_in")
            warm_out = dram.tile([1, H], BF16, name="warm_out")
            nc.gpsimd.collective_compute(
                "ReduceScatter", mybir.AluOpType.add,
                replica_groups=GROUPS,
                ins=[warm_in.opt()], outs=[warm_out.opt()])

            # per-chunk tensors (separate tiles -> no false dependencies)
            xT = [pw.tile([128, KT, CK], BF16, name=f"xT{j}")
                  for j in range(NQ)]
            QTt = [pw.tile([128, 2, CK], BF16, name=f"QT{j}")
                   for j in range(NQ)]
            KTt = [pw.tile([128, 2, CK], BF16, name=f"KT{j}")
                   for j in range(NQ)]
            V4 = [pw.tile([128, 4, HPC, DK + 1], BF16, name=f"V4{j}")
                  for j in range(NQ)]
            for j in range(NQ):
                nc.gpsimd.memset(V4[j], 1.0)  # ones col (rest overwritten)

            # ---------------- filler unit machinery ----------------
            # Each unit: (kind, chunk, cost_ns, closure).  kinds 't'/'q'
            # must be flushed before the first attention step of their
            # chunk; 'c' (c_proj) units are flexible.
            mq = deque()
            oq = deque()
            tr_rr = [0]

            def tr_evac(dst, src):
                # ACT for half: these run in chunk-start windows where the
                # exp pipeline has no backlog
                if tr_rr[0] % 2 == 0:
                    nc.scalar.copy(dst, src)
                else:
                    nc.vector.tensor_copy(dst, src)
                tr_rr[0] += 1

            def push_transpose_chunk(j):
                """x^T for chunk j: 4 xs DMAs now, 8 matmul-form units."""
                xs_tiles = []
                for sl in range(4):
                    si = 4 * j + sl
                    xs = pa.tile([128, H], BF16, tag="xs", bufs=4,
                                 name=f"xs{si}")
                    nc.sync.dma_start(out=xs,
                                      in_=xb[si * 128:(si + 1) * 128, :])
                    xs_tiles.append(xs)
                for sl in range(4):
                    for kh in range(2):
                        def tunit(j=j, sl=sl, kh=kh, xs=xs_tiles[sl]):
                            ptp = psum.tile([128, 512], F32, tag="pf", bufs=2,
                                            name=f"ptp{j}_{sl}_{kh}")
                            for i in range(4):
                                k = 4 * kh + i
                                nc.tensor.matmul(
                                    ptp[:, i * 128:(i + 1) * 128],
                                    xs[:, k * 128:(k + 1) * 128],
                                    ident, start=True, stop=True)
                            src = ptp[:, :].rearrange("p (k f) -> p k f", k=4)
                            dst = xT[j][:, 4 * kh:4 * kh + 4,
                                        sl * 128:(sl + 1) * 128]
                            tr_evac(dst, src)
                        mq.append((350, tunit))

            def push_qkv_chunk(j):
                for ti, (wt, dst, bias) in enumerate(
                        ((wq_sb, QTt[j], bq_sb), (wk_sb, KTt[j], bk_sb))):
                    for half in range(2):
                        pqs = {}

                        def qunit_a(j=j, wt=wt, half=half, pqs=pqs, ti=ti):
                            pq = psum.tile([128, 512], F32, tag="pf", bufs=2,
                                           name=f"pq{j}_{ti}_{half}")
                            pqs['t'] = pq
                            for k in range(4):
                                nc.tensor.matmul(
                                    pq,
                                    wt[:, k, half * 128:(half + 1) * 128],
                                    xT[j][:, k, :],
                                    start=(k == 0), stop=False)

                        def qunit_b(j=j, ti=ti, wt=wt, dst=dst, bias=bias,
                                    half=half, pqs=pqs):
                            pq = pqs.pop('t')
                            for k in range(4, KT):
                                nc.tensor.matmul(
                                    pq,
                                    wt[:, k, half * 128:(half + 1) * 128],
                                    xT[j][:, k, :],
                                    start=False, stop=(k == KT - 1))
                            if ti == 0:
                                nc.scalar.activation(
                                    dst[:, half, :], pq,
                                    mybir.ActivationFunctionType.Identity,
                                    bias=bias[:, half:half + 1])
                            else:
                                nc.vector.tensor_scalar_add(
                                    dst[:, half, :], pq,
                                    bias[:, half:half + 1])
                        mq.append((900, qunit_a))
                        mq.append((900, qunit_b))
                for sp in range(2):  # si pairs (0,1), (2,3)
                    pvs = {}

                    def vunit_a(j=j, sp=sp, pvs=pvs):
                        pv = psum.tile([128, 512], F32, tag="pf", bufs=2,
                                       name=f"pv{j}_{sp}")
                        pvs['t'] = pv
                        sl = 2 * sp
                        for k in range(KT):
                            nc.tensor.matmul(
                                pv[:, 0:256],
                                xT[j][:, k, sl * 128:(sl + 1) * 128],
                                wv_sb[:, k, :],
                                start=(k == 0), stop=(k == KT - 1))

                    def vunit_b(j=j, sp=sp, pvs=pvs):
                        pv = pvs.pop('t')
                        sl = 2 * sp + 1
                        for k in range(KT):
                            nc.tensor.matmul(
                                pv[:, 256:512],
                                xT[j][:, k, sl * 128:(sl + 1) * 128],
                                wv_sb[:, k, :],
                                start=(k == 0), stop=(k == KT - 1))
                        pv_v = pv[:, :].rearrange("p (s h d) -> p s h d",
                                                  s=2, h=HPC)
                        bv_h = bv_bc[:, :].rearrange("p (h d) -> p h d",
                                                     h=HPC)
                        nc.vector.tensor_add(
                            V4[j][:, 2 * sp:2 * sp + 2, :, 0:DK],
                            pv_v, _ins_bcast(bv_h, 1, 2))
                    mq.append((900, vunit_a))
                    mq.append((900, vunit_b))

            def push_cproj_chunk(j, normo):
                last = j == NQ - 1
                no = no_t[j]
                ps_t = {}
                for qb in range(4):
                    for nh in range(2):
                        def cunit(j=j, qb=qb, nh=nh, last=last, no=no,
                                  ps_t=ps_t):
                            pt = psum.tile([128, 512], F32, tag="pf", bufs=2,
                                           name=f"pt{j}_{qb}_{nh}")
                            for p in range(2):
                                nc.tensor.matmul(
                                    pt,
                                    no[qb][p][:, :],
                                    wp_sb[:, p, nh * 512:(nh + 1) * 512],
                                    start=(p == 0), stop=(p == 1))
                            ps = pa.tile([128, 512], BF16, tag="ps", bufs=9,
                                         name=f"ps{j}_{qb}_{nh}")
                            if last:
                                nc.scalar.copy(ps, pt)
                            else:
                                nc.vector.tensor_copy(ps, pt)
                            ps_t[(qb, nh)] = ps
                        oq.append((step_no[0] + 9, 500, cunit))

                def cdunit(j=j, ps_t=ps_t):
                    # all 8 partial DMAs in one burst AFTER the evacs have
                    # completed: the sync-queue triggers never block, and
                    # the RS trigger waits only on the DMA semaphores
                    for qb in range(4):
                        for nh in range(2):
                            nc.sync.dma_start(
                                out=partial[j][qb * 128:(qb + 1) * 128,
                                               nh * 512:(nh + 1) * 512],
                                in_=ps_t.pop((qb, nh)))
                    nc.gpsimd.collective_compute(
                        "ReduceScatter", mybir.AluOpType.add,
                        replica_groups=GROUPS,
                        ins=[partial[j].opt()],
                        outs=[rsout[j].opt()])
                    nc.sync.dma_start(out=outs[j][:, :], in_=rsout[j][:, :])
                oq.append((step_no[0] + 17, 300, cdunit))

            # ---------------- attention step machinery ----------------
            normo_h = {}   # (j, pair) -> (cbx, rbc)
            no_t = {}      # j -> {qb: [no_pair0, no_pair1]}
            pav_h = {}     # (j, pair) -> (pav_e, pav_o)

            def emit_scores(j, pair, ki):
                jk, kb = divmod(ki, 4)
                off = max(0, 128 * ki - CK * j)
                npp = CK - off
                sg = psum.tile([128, 2, CK], F32, tag="sg", bufs=2,
                               name=f"sg{j}_{pair}_{ki}")
                for half in range(2):
                    base = 64 * half
                    nc.tensor.matmul(
                        sg[:, half, off:CK],
                        KTt[jk][base:base + DK, pair,
                                kb * 128:(kb + 1) * 128],
                        QTt[j][base:base + DK, pair, off:CK],
                        start=True, stop=True)
                ag = pa.tile([128, 2, CK], BF16, tag="A", bufs=4,
                             name=f"A{j}_{pair}_{ki}")
                nc.scalar.activation(
                    ag[:, :, off:CK], sg[:, :, off:CK],
                    mybir.ActivationFunctionType.Exp)
                if ki >= 4 * j:  # diagonal block: causal corner mask
                    av = ag[:, :, off:off + 128]
                    nc.vector.tensor_mul(av, av, _ins_bcast(tri[:, :], 1, 2))
                if DEBUG and (j, pair, ki) == (0, 0, 0):
                    nc.sync.dma_start(
                        out=dbg["dbg_ag"].ap().rearrange(
                            "p (h c) -> p h c", h=2),
                        in_=ag)
                return (j, pair, ki, ag, off)

            def emit_av(st):
                j, pair, ki, ag, off = st
                nblk = 4 * j + 4
                if ki == 0:
                    pav_h[(j, pair)] = tuple(
                        psum.tile([65, 512], F32, tag="pav", bufs=2,
                                  name=f"pav{j}_{pair}_{h}")
                        for h in range(2))
                jk, kb = divmod(ki, 4)
                for half in range(2):
                    nc.tensor.matmul(
                        pav_h[(j, pair)][half][:, off:CK],
                        V4[jk][:, kb, 2 * pair + half, :],
                        ag[:, half, off:CK],
                        start=(ki == 0), stop=(ki == nblk - 1))
                if ki == nblk - 1:
                    emit_norm(j, pair)

            def emit_norm(j, pair):
                pav_e, pav_o = pav_h.pop((j, pair))
                # [0:65] copies carry the denominator row along with O^T,
                # releasing the pav PSUM banks in two DVE ops.
                cbx = [pa.tile([65, CK], BF16, tag=f"cbx{h}", bufs=2,
                               name=f"cbx{h}_{j}_{pair}") for h in range(2)]
                cpy = nc.scalar.copy if j == NQ - 1 else nc.vector.tensor_copy
                cpy(cbx[0][0:65, :], pav_e[0:65, :])
                cpy(cbx[1][0:65, :], pav_o[0:65, :])
                srec = pa.tile([128, 2, 4], BF16, tag="srec", bufs=2,
                               name=f"srec{j}_{pair}")
                nc.gpsimd.dma_start(out=srec[:, 0, :], in_=cbx[0][64:65, :])
                nc.gpsimd.dma_start(out=srec[:, 1, :], in_=cbx[1][64:65, :])
                with nc.allow_low_precision(
                        reason="bf16 recip of O(1e2) softmax sums"):
                    nc.vector.reciprocal(srec, srec)
                srr = [pa.tile([1, CK], BF16, tag=f"srr{h}", bufs=2,
                               name=f"srr{h}_{j}_{pair}") for h in range(2)]
                nc.gpsimd.dma_start(out=srr[0][0:1, :], in_=srec[:, 0, :])
                nc.gpsimd.dma_start(out=srr[1][0:1, :], in_=srec[:, 1, :])
                rbc = [pa.tile([64, CK], BF16, tag=f"rbc{h}", bufs=2,
                               name=f"rbc{h}_{j}_{pair}") for h in range(2)]
                nc.gpsimd.partition_broadcast(rbc[0][0:64, :], srr[0][0:1, :],
                                              channels=64)
                nc.gpsimd.partition_broadcast(rbc[1][0:64, :], srr[1][0:1, :],
                                              channels=64)
                # the normalize multiplies happen lazily in an oq unit,
                # long after this chain has drained
                no = no_t.setdefault(j, {})

                def nunit(no=no, cbx=cbx, rbc=rbc, j=j, pair=pair):
                    for qb in range(4):
                        qs = slice(qb * 128, (qb + 1) * 128)
                        t = no.setdefault(qb, [None, None])
                        t[pair] = pa.tile([128, 128], BF16, tag=f"no{pair}",
                                          bufs=8, name=f"no{pair}_{j}_{qb}")
                        nc.vector.tensor_mul(
                            t[pair][0:64, :], cbx[0][0:64, qs], rbc[0][:, qs])
                        nc.vector.tensor_mul(
                            t[pair][64:128, :], cbx[1][0:64, qs],
                            rbc[1][:, qs])
                oq.append((step_no[0] + 8, 300, nunit))
                normo_h[(j, pair)] = (cbx, rbc)
                if DEBUG and j == 0 and pair == 0:
                    nc.sync.dma_start(out=dbg["dbg_comb"][0:64, :],
                                      in_=cbx[0][0:64, :])
                    nc.sync.dma_start(out=dbg["dbg_comb"][64:128, :],
                                      in_=cbx[1][0:64, :])
                    nc.sync.dma_start(out=dbg["dbg_srow"][0:1, :],
                                      in_=cbx[0][64:65, :])
                    nc.sync.dma_start(out=dbg["dbg_srow"][1:2, :],
                                      in_=cbx[1][64:65, :])
                    nc.sync.dma_start(out=dbg["dbg_rbc"][0:64, :], in_=rbc[0])
                    nc.sync.dma_start(out=dbg["dbg_rbc"][64:128, :],
                                      in_=rbc[1])
                    nc.sync.dma_start(out=dbg["dbg_no"][:, :], in_=no)
                if DEBUG and j == 2 and pair == 0:
                    nc.sync.dma_start(out=dbg["dbg_no2"][:, :], in_=no)
                if pair == 1:
                    push_cproj_chunk(j, [normo_h.pop((j, 0)),
                                         normo_h.pop((j, 1))])

            # ---------------- the fused stream ----------------
            steps = [(j, pair, ki)
                     for j in range(NQ)
                     for pair in range(2)
                     for ki in range(4 * j + 4)]

            push_transpose_chunk(0)
            push_qkv_chunk(0)

            pending = deque()   # scores emitted, AV not yet (depth 2)
            cur_chunk = [-1]
            step_no = [0]
            mquota = [0.0]
            mbudget = [0.0]

            for (j, pair, ki) in steps:
                if j != cur_chunk[0]:
                    cur_chunk[0] = j
                    # the current chunk's x^T/QKV must be fully emitted
                    # before its first scores instruction
                    while mq:
                        cost, fn = mq.popleft()
                        fn()
                    if j + 1 < NQ:
                        push_transpose_chunk(j + 1)
                        push_qkv_chunk(j + 1)
                    # ration the next chunk's mandatory work over this
                    # chunk's steps (front-loaded slightly: x0.75 steps)
                    nsteps = 2 * (4 * j + 4)
                    mquota[0] = sum(u[0] for u in mq) / max(nsteps * 0.75, 1)
                    mbudget[0] = 0.0
                pending.append(emit_scores(j, pair, ki))
                mbudget[0] += mquota[0]
                while mq and mbudget[0] >= mq[0][0] * 0.5:
                    cost, fn = mq.popleft()
                    mbudget[0] -= cost
                    fn()
                step_no[0] += 1
                if oq and step_no[0] >= oq[0][0]:
                    # trickle one c_proj unit per step, delayed so the
                    # norm chain has completed by the first pop
                    _, cost, fn = oq.popleft()
                    fn()
                if len(pending) > 2:
                    emit_av(pending.popleft())

            while pending:
                emit_av(pending.popleft())
            while mq:
                cost, fn = mq.popleft()
                fn()
            while oq:
                _, cost, fn = oq.popleft()
                fn()

            if DEBUG:
                nc.sync.dma_start(
                    out=dbg["dbg_qt"].ap().rearrange("p (h c) -> p h c", h=2),
                    in_=QTt[0])
                nc.sync.dma_start(
                    out=dbg["dbg_kt"].ap().rearrange("p (h c) -> p h c", h=2),
                    in_=KTt[0])
                nc.sync.dma_start(
                    out=dbg["dbg_v4"].ap().rearrange(
                        "p (s h d) -> p s h d", s=4, h=HPC),
                    in_=V4[0])
                nc.sync.dma_start(out=dbg["dbg_partial"][:, :],
                                  in_=partial[0][:, :])

    nc.compile()
    return nc


def make_in_maps(x, w_attn, b_attn, w_proj):
    import ml_dtypes
    bf = ml_dtypes.bfloat16
    x = np.asarray(x, dtype=np.float32)
    w_attn = np.asarray(w_attn, dtype=np.float32)
    b_attn = np.asarray(b_attn, dtype=np.float32)
    w_proj = np.asarray(w_proj, dtype=np.float32)
    in_maps = []
    for c in range(NCORES):
        b, g = divmod(c, 4)
        h0 = g * HPC
        cs = slice(h0 * DK, h0 * DK + CW)
        wpc = np.ascontiguousarray(w_proj[cs, :])          # [256, H]
        wp_pair = np.ascontiguousarray(
            wpc.reshape(2, 128, H).transpose(1, 0, 2).reshape(128, 2 * H))
        in_maps.append({
            "xb": np.ascontiguousarray(x[b]).astype(bf),
            # fold the 1/sqrt(DK)=2^-3 score scale into Wq/bq (exact in fp32)
            "wq": (np.ascontiguousarray(w_attn[:, cs])
                   * np.float32(0.125)).astype(bf),
            "wk": np.ascontiguousarray(w_attn[:, H:][:, cs]).astype(bf),
            "wv": np.ascontiguousarray(w_attn[:, 2 * H:][:, cs]).astype(bf),
            "wp": wp_pair.astype(bf),
            "bq": np.ascontiguousarray(b_attn[cs]) * np.float32(0.125),
            "bk": np.ascontiguousarray(b_attn[H:][cs]),
            "bv": np.ascontiguousarray(b_attn[2 * H:][cs]),
        })
    return in_maps


_NC = None


def kernel(x, w_attn, b_attn, w_proj, b_proj):
    global _NC
    if _NC is None:
        _NC = build_nc()

    b_proj = np.asarray(b_proj, dtype=np.float32)
    in_maps = make_in_maps(x, w_attn, b_attn, w_proj)
    res = run_bass_kernel_spmd(_NC, in_maps, core_ids=list(range(NCORES)))

    outp = np.empty((B, S, H), dtype=np.float32)
    for c in range(NCORES):
        b, g = divmod(c, 4)
        for j in range(NQ):
            o = res.results[c][f"out{j}"].astype(np.float32)
            outp[b, j * CK + g * 128: j * CK + (g + 1) * 128, :] = o
    outp += b_proj  # row-broadcast add, exact
    return outp
